# revision 9
# baseline (speedup 1.0000x reference)
# Multi-scale deformable attention kernel for TRN2 (per-core: one batch element).
#
# v2: per-level M2 maps built smallest-level-first so gathers overlap the
# level-0 build; entry pairs are folded two-per-partition in SBUF via
# permutation matmuls so M2 writes are 512B-contiguous (full DMA bandwidth,
# ~4x fewer descriptors than the per-(pos,head) strided writes of v1).
#
# M2 layout (per level l, per head h): entry e = 64 f32 =
#   [v_h(LS_l + e - PAD_l), v_h(LS_l + e - PAD_l + W_l)]   (v zero-padded
# outside [0, L)). A 512B gather at entry (y0,x0) = PAD_l + y0*W_l + x0
# returns all 4 bilinear corners. Phase 2 runs three gather passes
# (levels 2, 1, 0), accumulating per-chunk O tiles, then the Wout matmul.
import sys

sys.path.insert(0, "/opt/trn_rl_repo")
import numpy as np

import concourse.bacc as bacc
import concourse.bass as bass
import concourse.mybir as mybir
import concourse.tile as tile
import bass_rust
from concourse.alu_op_type import AluOpType
from concourse.masks import make_identity

F32 = mybir.dt.float32
BF16 = mybir.dt.bfloat16
I32 = mybir.dt.int32
I16 = mybir.dt.int16
AX = mybir.AxisListType
AF = mybir.ActivationFunctionType

SHAPES = ((100, 168), (50, 84), (25, 42))
NH, NL, NP = 8, 3, 4
P8 = 2 * NP              # 8 sampling points per (head, level)
C, D = 256, 32
W_ = [w for h, w in SHAPES]
H_ = [h for h, w in SHAPES]
LVL_START = [0, 16800, 21000]
L = 22050
PAD_L = [w + 2 for w in W_]                       # 170, 86, 44
# entries per level, rounded up to 256 (one fold round = 256 entries)
NENT_L = [-(-(PAD_L[l] + H_[l] * W_[l] + 2) // 256) * 256 for l in range(NL)]
NR_L = [n // 256 for n in NENT_L]                 # 67, 17, 5 rounds
HS_L = [n * 2 * D for n in NENT_L]                # head stride in f32
LQ = 1700
LQP = 1792               # 14 chunks of 128
NCH = LQP // 128
SLOTS = NH * NL * P8     # 192 (h,l,p) combos per query


def build_program(num_cores=8, dbg=False, mode='full'):
    nc = bacc.Bacc("TRN2", target_bir_lowering=False, debug=False,
                   num_devices=num_cores, num_swdge_queues=4)
    value = nc.dram_tensor("value", [L, 2 * C], F32, kind="ExternalInput")
    query = nc.dram_tensor("query", [LQP, C], F32, kind="ExternalInput")
    refp = nc.dram_tensor("refp", [LQP, 4 * NL], F32, kind="ExternalInput")
    consts = nc.dram_tensor("consts", [6 * SLOTS], F32, kind="ExternalInput")
    Wv = nc.dram_tensor("Wv", [2 * C, C], F32, kind="ExternalInput")
    bvr = nc.dram_tensor("bvr", [1, C], F32, kind="ExternalInput")
    Woff = nc.dram_tensor("Woff", [C, SLOTS * 2], F32, kind="ExternalInput")
    boffr = nc.dram_tensor("boffr", [1, SLOTS * 2], F32, kind="ExternalInput")
    Watt = nc.dram_tensor("Watt", [C, 96], F32, kind="ExternalInput")
    battr = nc.dram_tensor("battr", [1, 96], F32, kind="ExternalInput")
    Wout = nc.dram_tensor("Wout", [C, C], F32, kind="ExternalInput")
    boutr = nc.dram_tensor("boutr", [1, C], F32, kind="ExternalInput")
    out = nc.dram_tensor("out", [LQP, C], F32, kind="ExternalOutput")
    m2l = [nc.dram_tensor(f"m2{l}", [NH * HS_L[l]], F32, kind="Internal")
           for l in range(NL)]

    from contextlib import ExitStack
    with tile.TileContext(nc) as tc:
      with ExitStack() as ctx:
        # ---------------- constant / parameter loads ----------------
        wp = ctx.enter_context(tc.tile_pool(name="wp", bufs=1))
        ident = wp.tile([128, 128], F32)
        make_identity(nc, ident[:])
        wv_t = [wp.tile([128, C], F32, tag=f"wv{k}", name=f"wv{k}") for k in range(4)]
        for k in range(4):
            nc.sync.dma_start(wv_t[k][:], Wv[128 * k:128 * (k + 1), :])
        woff_t = [wp.tile([128, SLOTS * 2], F32, tag=f"woff{k}", name=f"woff{k}") for k in range(2)]
        watt_t = [wp.tile([128, 96], F32, tag=f"watt{k}", name=f"watt{k}") for k in range(2)]
        wout_t = [wp.tile([128, C], F32, tag=f"wout{k}", name=f"wout{k}") for k in range(2)]
        for k in range(2):
            nc.sync.dma_start(woff_t[k][:], Woff[128 * k:128 * (k + 1), :])
            nc.sync.dma_start(watt_t[k][:], Watt[128 * k:128 * (k + 1), :])
            nc.sync.dma_start(wout_t[k][:], Wout[128 * k:128 * (k + 1), :])
        bv_t = wp.tile([1, C], F32)
        boff_t = wp.tile([1, SLOTS * 2], F32)
        batt_t = wp.tile([1, 96], F32)
        bout_t = wp.tile([1, C], F32)
        nc.sync.dma_start(bv_t[:], bvr[:])
        nc.sync.dma_start(boff_t[:], boffr[:])
        nc.sync.dma_start(batt_t[:], battr[:])
        nc.sync.dma_start(bout_t[:], boutr[:])
        ones_t = wp.tile([1, 128], F32)
        nc.gpsimd.memset(ones_t[:], 1.0)
        cst_row = wp.tile([1, 6 * SLOTS], F32)
        nc.sync.dma_start(cst_row[:], consts.ap().unsqueeze(0))
        cst = wp.tile([128, 6 * SLOTS], F32)
        nc.gpsimd.partition_broadcast(cst[:], cst_row[:])
        WT = cst[:, 0 * SLOTS:1 * SLOTS]
        W1 = cst[:, 1 * SLOTS:2 * SLOTS]
        W2 = cst[:, 2 * SLOTS:3 * SLOTS]
        H1 = cst[:, 3 * SLOTS:4 * SLOTS]
        H2 = cst[:, 4 * SLOTS:5 * SLOTS]
        BS = cst[:, 5 * SLOTS:6 * SLOTS]
        # bf16 copies: Wv, bias, ones, even/odd extraction matrices
        wvb = [wp.tile([128, C], BF16, tag=f"wvb{k}", name=f"wvb{k}") for k in range(4)]
        for k in range(4):
            nc.vector.tensor_copy(wvb[k][:], wv_t[k][:])
        bvb = wp.tile([1, C], BF16)
        nc.vector.tensor_copy(bvb[:], bv_t[:])
        onesb = wp.tile([1, 128], BF16)
        nc.gpsimd.memset(onesb[:], 1.0)
        # MB[par] = [zeros64 | M_par | zeros64], M_par[p,j] = 1 iff p = 2j+par.
        # Slices give stride-2 row extraction with implicit zero masking at
        # tile boundaries (PE psum writes must start at partition 0/64).
        iv = ident[:].rearrange("p (j t) -> p j t", t=2)
        MB = [wp.tile([128, 192], BF16, tag=f"MB{p}", name=f"MB{p}") for p in range(2)]
        for p in range(2):
            nc.vector.memset(MB[p][:], 0.0)
            nc.vector.tensor_copy(MB[p][:, 64:128], iv[:, :, p])

        # ---------------- pools ----------------
        p1 = ctx.enter_context(tc.tile_pool(name="p1", bufs=3))
        vrp = ctx.enter_context(tc.tile_pool(name="vrp", bufs=8))
        ptd = ctx.enter_context(tc.tile_pool(name="ptd", bufs=2))
        psum = ctx.enter_context(tc.tile_pool(name="psum", bufs=1, space="PSUM"))
        p2 = ctx.enter_context(tc.tile_pool(name="p2", bufs=1))
        pwp = ctx.enter_context(tc.tile_pool(name="pwp", bufs=1))
        pg = ctx.enter_context(tc.tile_pool(name="pg", bufs=3))

        # ---------------- phase 1: per-level M2 build ----------------
        vtiles = {}

        def emit_proj(t):
            # project value rows [128t, 128t+128) -> bf16 v tile in the ring
            p0 = 128 * t
            vt = vrp.tile([128, C], BF16, tag="vring", name=f"vr{t}")
            rlo, rhi = max(0, -p0), min(128, L - p0)
            if rlo > 0 or rhi < 128:
                nc.vector.memset(vt[:], 0.0)
            if rhi > rlo:
                nr = rhi - rlo
                vin = p1.tile([128, 2 * C], F32, tag="vin")
                nc.sync.dma_start(vin[rlo:rhi, :], value[p0 + rlo:p0 + rhi, :])
                vT = p1.tile([128, 2 * C], BF16, tag="vT")
                for k in range(4):
                    pt = psum.tile([128, 128], F32, tag="tp", bufs=2, name="ptp")
                    nc.tensor.transpose(pt[:, 0:nr], vin[rlo:rhi, 128 * k:128 * (k + 1)],
                                        ident[0:nr, 0:nr])
                    nc.scalar.copy(vT[:, 128 * k:128 * k + nr], pt[:, 0:nr])
                ps = psum.tile([128, SLOTS * 2], F32, tag="mm", bufs=2, name="pmm")
                for k in range(4):
                    nc.tensor.matmul(ps[rlo:rhi, 0:C], vT[:, 128 * k:128 * k + nr],
                                     wvb[k][:], start=(k == 0), stop=False)
                nc.tensor.matmul(ps[rlo:rhi, 0:C], onesb[:, 0:nr], bvb[:],
                                 start=False, stop=True)
                nc.scalar.copy(vt[rlo:rhi, :], ps[rlo:rhi, 0:C])
            vtiles[t] = vt

        def emit_level_build(l):
            W, PADl, LS = W_[l], PAD_L[l], LVL_START[l]
            vtiles.clear()
            nxt = (LS - PADl) // 128
            for r in range(NR_L[l]):
                e0 = 256 * r
                need_hi = (LS - PADl + e0 + 255 + W + 1) // 128
                while nxt <= need_hi:
                    emit_proj(nxt)
                    nxt += 1
                # extraction: es cols [0:256)=E0 [256:512)=E1 [512:768)=S0 [768:1024)=S1
                es = psum.tile([128, 4 * C], F32, tag="es", bufs=2, name="es")
                for pi, (par, sh) in enumerate(
                        ((0, -PADl), (1, -PADl), (0, W - PADl), (1, W - PADl))):
                    base = LS + e0 + par + sh
                    for jb in (0, 64):
                        P0 = base + 2 * jb
                        t = P0 // 128
                        s0 = P0 - 128 * t
                        colA, par2 = s0 // 2, s0 % 2
                        dst = es[jb:jb + 64, 256 * pi:256 * pi + 256]
                        if colA == 0:
                            nc.tensor.matmul(dst, MB[par2][:, 64:128], vtiles[t][:],
                                             start=True, stop=True)
                        else:
                            nc.tensor.matmul(dst, MB[par2][:, 64 + colA:128 + colA],
                                             vtiles[t][:], start=True, stop=False)
                            nc.tensor.matmul(dst, MB[par2][:, colA:colA + 64],
                                             vtiles[t + 1][:], start=False, stop=True)
                # assemble: partition j holds entries e0+2j, e0+2j+1 for all heads
                TD = ptd.tile([128, 1024], F32, tag="TD")
                TDv = TD[:].rearrange("p (h g d) -> p h g d", g=4, d=D)
                for gi, pi in enumerate((0, 2, 1, 3)):    # E0 S0 E1 S1
                    nc.scalar.copy(
                        TDv[:, :, gi, :],
                        es[:, 256 * pi:256 * pi + 256].rearrange("p (h d) -> p h d", d=D))
                for h in range(NH):
                    seg = m2l[l].ap()[h * HS_L[l] + e0 * 2 * D:
                                      h * HS_L[l] + (e0 + 256) * 2 * D]
                    nc.sync.dma_start(seg.rearrange("(p c) -> p c", c=128),
                                      TD[:, 128 * h:128 * (h + 1)])

        # ---------------- phase 2 pre-work (per chunk, M2-independent) ----
        wrp_t, coefx_t, O_t = {}, {}, {}

        def emit_prework(ch):
            q0 = ch * 128
            qin = p2.tile([128, C], F32, tag="qin", bufs=2)
            nc.sync.dma_start(qin[:], query[q0:q0 + 128, :])
            rp = p2.tile([128, 4 * NL], F32, tag="rp", bufs=2)
            nc.sync.dma_start(rp[:], refp[q0:q0 + 128, :])
            qT = p2.tile([128, 256], F32, tag="qT", bufs=2)
            for k in range(2):
                pt2 = psum.tile([128, 128], F32, tag="tp", bufs=2, name="pt2")
                nc.tensor.transpose(pt2[:], qin[:, 128 * k:128 * (k + 1)], ident[:])
                nc.scalar.copy(qT[:, 128 * k:128 * (k + 1)], pt2[:])
            # off = q @ Woff + boff   [128, 384]
            pso = psum.tile([128, SLOTS * 2], F32, tag="mm", bufs=2, name="pso")
            for k in range(2):
                nc.tensor.matmul(pso[:], qT[:, 128 * k:128 * (k + 1)], woff_t[k][:],
                                 start=(k == 0), stop=False)
            nc.tensor.matmul(pso[:], ones_t[:], boff_t[:], start=False, stop=True)
            off = p2.tile([128, SLOTS * 2], F32, tag="off", bufs=2)
            nc.scalar.copy(off[:], pso[:])
            # att = q @ Watt + batt -> per-head softmax over 12 -> aw [128, 96]
            psa = psum.tile([128, SLOTS * 2], F32, tag="mm", bufs=2, name="psa")
            for k in range(2):
                nc.tensor.matmul(psa[:, 0:96], qT[:, 128 * k:128 * (k + 1)], watt_t[k][:],
                                 start=(k == 0), stop=False)
            nc.tensor.matmul(psa[:, 0:96], ones_t[:], batt_t[:], start=False, stop=True)
            att = p2.tile([128, 96], F32, tag="att")
            nc.scalar.copy(att[:], psa[:, 0:96])
            rmax = p2.tile([128, 8], F32, tag="rmax")
            nc.vector.tensor_reduce(rmax[:], att[:].rearrange("q (h l) -> q h l", l=12), AX.X, AluOpType.max)
            nc.vector.tensor_tensor(att[:].rearrange("q (h l) -> q h l", l=12),
                                    att[:].rearrange("q (h l) -> q h l", l=12),
                                    rmax[:].unsqueeze(2).broadcast_to((128, 8, 12)), AluOpType.subtract)
            nc.scalar.activation(att[:], att[:], AF.Exp)
            rsum = p2.tile([128, 8], F32, tag="rsum")
            nc.vector.tensor_reduce(rsum[:], att[:].rearrange("q (h l) -> q h l", l=12), AX.X, AluOpType.add)
            nc.vector.reciprocal(rsum[:], rsum[:])
            aw = p2.tile([128, 96], F32, tag="aw")
            nc.vector.tensor_tensor(aw[:].rearrange("q (h l) -> q h l", l=12),
                                    att[:].rearrange("q (h l) -> q h l", l=12),
                                    rsum[:].unsqueeze(2).broadcast_to((128, 8, 12)), AluOpType.mult)

            # ---- sampling coords: X,Y [128, 192] in slot order s=(h,l,p8)
            X = p2.tile([128, SLOTS], F32, tag="X")
            Y = p2.tile([128, SLOTS], F32, tag="Y")
            for du in range(2):
                for xy in range(2):
                    T = (X if xy == 0 else Y)
                    for li in range(NL):
                        dst = T[:].rearrange("q (hl p) -> q hl p", p=P8)[:, li::NL, du * NP:(du + 1) * NP]
                        src0 = off[:].rearrange("q (hl pc) -> q hl pc", pc=16)[:, li::NL, 2 * du + xy:2 * du + xy + 13:4]
                        src1 = rp[:, 4 * li + 2 * du + xy].unsqueeze(1).unsqueeze(2).broadcast_to((128, NH, NP))
                        nc.vector.scalar_tensor_tensor(dst, src0, -0.5, src1, AluOpType.add, AluOpType.add)
            TX = p2.tile([128, SLOTS], F32, tag="TX")
            TY = p2.tile([128, SLOTS], F32, tag="TY")
            X0 = p2.tile([128, SLOTS], F32, tag="X0")
            Y0 = p2.tile([128, SLOTS], F32, tag="Y0")
            MAGIC = 12582912.0  # 1.5 * 2^23: (x+M)-M = round-to-nearest(x)
            nc.vector.tensor_scalar(TX[:], X[:], MAGIC, MAGIC, AluOpType.add, AluOpType.subtract)
            nc.vector.tensor_scalar(TY[:], Y[:], MAGIC, MAGIC, AluOpType.add, AluOpType.subtract)
            nc.vector.tensor_tensor(X0[:], TX[:], X[:], AluOpType.is_gt)
            nc.vector.tensor_tensor(Y0[:], TY[:], Y[:], AluOpType.is_gt)
            nc.vector.tensor_tensor(X0[:], TX[:], X0[:], AluOpType.subtract)  # floor(x)
            nc.vector.tensor_tensor(Y0[:], TY[:], Y0[:], AluOpType.subtract)
            nc.vector.tensor_tensor(TX[:], X[:], X0[:], AluOpType.subtract)   # frac
            nc.vector.tensor_tensor(TY[:], Y[:], Y0[:], AluOpType.subtract)
            UX = p2.tile([128, SLOTS], F32, tag="UX")
            UY = p2.tile([128, SLOTS], F32, tag="UY")
            nc.vector.tensor_tensor(UX[:], W1, X0[:], AluOpType.subtract)   # W-1-x0
            nc.vector.tensor_tensor(UY[:], H1, Y0[:], AluOpType.subtract)
            MX0 = p2.tile([128, SLOTS], F32, tag="MX0")
            MY0 = p2.tile([128, SLOTS], F32, tag="MY0")
            MX1 = p2.tile([128, SLOTS], F32, tag="MX1")
            MY1 = p2.tile([128, SLOTS], F32, tag="MY1")
            nc.vector.tensor_tensor(MX0[:], X0[:], UX[:], AluOpType.min)
            nc.vector.tensor_tensor(MY0[:], Y0[:], UY[:], AluOpType.min)
            UX2 = p2.tile([128, SLOTS], F32, tag="UX2")
            UY2 = p2.tile([128, SLOTS], F32, tag="UY2")
            nc.vector.tensor_tensor(UX2[:], W2, X0[:], AluOpType.subtract)
            nc.vector.tensor_tensor(UY2[:], H2, Y0[:], AluOpType.subtract)
            nc.vector.scalar_tensor_tensor(MX1[:], X0[:], 1.0, UX2[:], AluOpType.add, AluOpType.min)
            nc.vector.scalar_tensor_tensor(MY1[:], Y0[:], 1.0, UY2[:], AluOpType.add, AluOpType.min)
            awsx = p2.tile([128, SLOTS], F32, tag="awsx")
            axv = awsx[:].rearrange("q (hl dp) -> q hl dp", dp=P8)
            avv = aw[:].rearrange("q (hl p) -> q hl p", p=NP)
            nc.vector.tensor_copy(axv[:, :, 0:NP], avv)
            nc.vector.tensor_copy(axv[:, :, NP:P8], avv)
            A = p2.tile([128, SLOTS], F32, tag="A")    # 1-tx
            B = p2.tile([128, SLOTS], F32, tag="B")    # 1-ty
            nc.vector.tensor_scalar(A[:], TX[:], -1.0, 1.0, AluOpType.mult, AluOpType.add)
            nc.vector.tensor_scalar(B[:], TY[:], -1.0, 1.0, AluOpType.mult, AluOpType.add)
            coefx = pwp.tile([128, SLOTS * 4], F32, tag=f"coefx{ch}", name=f"coefx{ch}")
            cxv = coefx[:].rearrange("q (s c) -> q s c", c=4)
            vv = p2.tile([128, SLOTS], F32, tag="vv")
            wgt = p2.tile([128, SLOTS], F32, tag="wgt")
            for (ci, mx, my, wa, wb) in ((0, MX0, MY0, A, B), (1, MX0, MY1, A, TY),
                                         (2, MX1, MY0, TX, B), (3, MX1, MY1, TX, TY)):
                nc.vector.tensor_tensor(vv[:], mx[:], my[:], AluOpType.min)
                nc.vector.scalar_tensor_tensor(vv[:], vv[:], 0.0, awsx[:], AluOpType.is_ge, AluOpType.mult)
                nc.vector.tensor_tensor(wgt[:], wa[:], wb[:], AluOpType.mult)
                nc.vector.tensor_tensor(cxv[:, :, ci], wgt[:], vv[:], AluOpType.mult)
            # entry idx = BS + y0c*WT + x0c  (clamped)
            X0C = p2.tile([128, SLOTS], F32, tag="X0C")
            Y0C = p2.tile([128, SLOTS], F32, tag="Y0C")
            nc.vector.tensor_scalar(X0C[:], X0[:], -1.0, None, AluOpType.max)
            nc.vector.tensor_tensor(X0C[:], X0C[:], W1, AluOpType.min)
            nc.vector.tensor_scalar(Y0C[:], Y0[:], -1.0, None, AluOpType.max)
            nc.vector.tensor_tensor(Y0C[:], Y0C[:], H1, AluOpType.min)
            IDXF = p2.tile([128, SLOTS], F32, tag="IDXF")
            nc.vector.tensor_tensor(IDXF[:], Y0C[:], WT, AluOpType.mult)
            nc.vector.tensor_tensor(IDXF[:], IDXF[:], X0C[:], AluOpType.add)
            nc.vector.tensor_tensor(IDXF[:], IDXF[:], BS, AluOpType.add)
            IDX32 = p2.tile([128, SLOTS], I32, tag="IDX32")
            nc.vector.tensor_copy(IDX32[:], IDXF[:])
            IDX16 = p2.tile([128, SLOTS], I16, tag="IDX16")
            nc.vector.tensor_copy(IDX16[:], IDX32[:])
            # fold to wrapped layout (see v1): wrp[p%16, S*8+j] = IDX16[16j+p%16, S]
            T16 = p2.tile([128, SLOTS], I16, tag="T16")
            nc.vector.stream_shuffle(T16[:], IDX16[:], [(i + 16) % 32 for i in range(32)])
            stage = p2.tile([128, SLOTS * 8], I16, tag="stage", bufs=2)
            nc.vector.memset(stage[0:32, :], 0)
            sv = stage[:].rearrange("p (s j) -> p s j", j=8)
            for k in range(4):
                nc.vector.tensor_copy(sv[0:16, :, 2 * k], IDX16[32 * k:32 * k + 16, :])
                nc.vector.tensor_copy(sv[0:16, :, 2 * k + 1], T16[32 * k:32 * k + 16, :])
            nc.vector.tensor_copy(stage[32:64, :], stage[0:32, :])
            nc.vector.tensor_copy(stage[64:96, :], stage[0:32, :])
            nc.vector.tensor_copy(stage[96:128, :], stage[0:32, :])
            wrp = pwp.tile([128, SLOTS * 8], I16, tag=f"wrp{ch}", name=f"wrp{ch}")
            nc.vector.stream_shuffle(wrp[:], stage[:], [i % 16 for i in range(32)])
            Ot = pwp.tile([128, C], F32, tag=f"O{ch}", name=f"O{ch}")
            wrp_t[ch], coefx_t[ch], O_t[ch] = wrp, coefx, Ot

        # ---------------- emission ----------------
        emit_level_build(2)
        for ch in range(4):
            emit_prework(ch)
        emit_level_build(1)
        for ch in range(4, NCH):
            emit_prework(ch)
        emit_level_build(0)

        # ---------------- gather passes ----------------
        for l in (2, 1, 0):
            for ch in range(NCH):
                for h in range(NH):
                    s0 = (h * NL + l) * P8        # first slot of (h,l) group
                    G = pg.tile([128, P8 * 128], F32, tag="G")
                    m2ap = m2l[l].ap()
                    m2ap.ap = bass_rust.VecI64Pair([[2 * D, NENT_L[l] - 1], [1, 4 * D]])
                    m2ap.offset = h * HS_L[l]
                    if mode == 'nog':
                        nc.vector.memset(G[:], 0.01)
                    else:
                        nc.gpsimd.dma_gather(
                            G[:].rearrange("q (s e) -> q s e", e=128), m2ap,
                            wrp_t[ch][:, 8 * s0:8 * s0 + 64], P8 * 128, P8 * 128, 128,
                            elem_step=2 * D, queue_num=(h * NL + l) % 4,
                            single_packet=False)
                    TMP = pg.tile([128, P8 * 128], F32, tag="TMP")
                    cb = coefx_t[ch][:, 4 * s0:4 * s0 + 32].unsqueeze(2).broadcast_to((128, 32, 32))
                    nc.vector.tensor_tensor(TMP[:].rearrange("q (sc c) -> q sc c", c=32),
                                            G[:].rearrange("q (sc c) -> q sc c", c=32),
                                            cb, AluOpType.mult)
                    if l == 2:
                        nc.vector.tensor_reduce(O_t[ch][:, D * h:D * (h + 1)],
                                                TMP[:].rearrange("q (sc c) -> q c sc", c=32),
                                                AX.X, AluOpType.add)
                    else:
                        Or = pg.tile([128, D], F32, tag="Or")
                        nc.vector.tensor_reduce(Or[:],
                                                TMP[:].rearrange("q (sc c) -> q c sc", c=32),
                                                AX.X, AluOpType.add)
                        nc.vector.tensor_tensor(O_t[ch][:, D * h:D * (h + 1)],
                                                O_t[ch][:, D * h:D * (h + 1)], Or[:],
                                                AluOpType.add)

        # ---------------- out = O @ Wout + bout ----------------
        for ch in range(NCH):
            q0 = ch * 128
            OT = p2.tile([128, 256], F32, tag="OT", bufs=2)
            for k in range(2):
                pt3 = psum.tile([128, 128], F32, tag="tp", bufs=2, name="pt3")
                nc.tensor.transpose(pt3[:], O_t[ch][:, 128 * k:128 * (k + 1)], ident[:])
                nc.scalar.copy(OT[:, 128 * k:128 * (k + 1)], pt3[:])
            pso2 = psum.tile([128, SLOTS * 2], F32, tag="mm", bufs=2, name="pso2")
            for k in range(2):
                nc.tensor.matmul(pso2[:, 0:C], OT[:, 128 * k:128 * (k + 1)], wout_t[k][:],
                                 start=(k == 0), stop=False)
            nc.tensor.matmul(pso2[:, 0:C], ones_t[:], bout_t[:], start=False, stop=True)
            OO = p2.tile([128, C], F32, tag="OO", bufs=2)
            nc.scalar.copy(OO[:], pso2[:, 0:C])
            nc.sync.dma_start(out[q0:q0 + 128, :], OO[:])

    nc.finalize()
    return nc


# ---------------- host-side wrapper ----------------
def prep_core_inputs(inputs, b):
    q = np.zeros((LQP, C), np.float32)
    q[:LQ] = inputs["query"][b]
    rl = inputs["ref_l"][b].transpose(0, 2, 1, 3).reshape(LQ, NL, 2)
    rr = inputs["ref_r"][b].transpose(0, 2, 1, 3).reshape(LQ, NL, 2)
    norm = np.array([[w, h] for h, w in SHAPES], np.float32)
    rp = np.zeros((LQP, NL, 4), np.float32)
    rp[:LQ, :, 0:2] = rl * norm
    rp[:LQ, :, 2:4] = rr * norm
    slot_l = np.repeat(np.tile(np.arange(NL), NH), P8).astype(np.int32)  # slot -> level
    Wl = np.array(W_, np.float32)[slot_l]
    Hl = np.array(H_, np.float32)[slot_l]
    Bs = np.array(PAD_L, np.float32)[slot_l]       # per-level local base
    consts = np.concatenate([Wl, Wl - 1, Wl - 2, Hl - 1, Hl - 2, Bs]).astype(np.float32)
    return {
        "value": np.ascontiguousarray(inputs["value"][b]),
        "query": q,
        "refp": rp.reshape(LQP, 4 * NL).astype(np.float32),
        "consts": consts,
        "Wv": inputs["Wv"], "bvr": inputs["bv"][None, :],
        "Woff": inputs["Woff"], "boffr": inputs["boff"][None, :],
        "Watt": inputs["Watt"], "battr": inputs["batt"][None, :],
        "Wout": inputs["Wout"], "boutr": inputs["bout"][None, :],
    }


LAST_EXEC_NS = None


def kernel(**inputs):
    global LAST_EXEC_NS
    import os
    from concourse.bass_utils import run_bass_kernel_spmd
    nc = build_program(num_cores=8)
    in_maps = [prep_core_inputs(inputs, b) for b in range(8)]
    trace = bool(int(os.environ.get("DKA_TRACE", "0")))
    tdir = None
    if trace:
        tdir = "/tmp/dka_trace"
        os.makedirs(tdir, exist_ok=True)
    res = run_bass_kernel_spmd(nc, in_maps, core_ids=list(range(8)), trace=trace,
                               tmpdir=tdir)
    LAST_EXEC_NS = res.exec_time_ns
    return np.stack([res.results[b]["out"][:LQ] for b in range(8)], 0)


# revision 11
# speedup vs baseline: 1.2371x; 1.2371x over previous
# Multi-scale deformable attention kernel for TRN2 (per-core: one batch element).
#
# v3: per-level bf16 quad-corner M2 maps. Entry e of level l, head h =
#   [v_h(p), v_h(p+1), v_h(p+W), v_h(p+W+1)] (4x32 bf16 = 256B), p = LS_l+e-PAD_l
# so ONE 256B gather returns all 4 bilinear corners (order TL,TR,BL,BR).
# Build is fold-2: partition j holds entries 2j,2j+1 -> 512B-contiguous
# M2 writes at full DMA bandwidth. Levels built smallest-first; gather
# passes run per level (2,1,0) so the level-0 build overlaps the level-2/1
# gathers. Pool-engine descriptor generation (~5ns/idx) is the critical
# resource; the per-chunk coord/coef/idx vector work is emitted inside the
# level-2 pass so gather-buffer recycling never queues behind it.
import sys

sys.path.insert(0, "/opt/trn_rl_repo")
import numpy as np

import concourse.bacc as bacc
import concourse.bass as bass
import concourse.mybir as mybir
import concourse.tile as tile
import bass_rust
from concourse.alu_op_type import AluOpType
from concourse.masks import make_identity

F32 = mybir.dt.float32
BF16 = mybir.dt.bfloat16
I32 = mybir.dt.int32
I16 = mybir.dt.int16
AX = mybir.AxisListType
AF = mybir.ActivationFunctionType

SHAPES = ((100, 168), (50, 84), (25, 42))
NH, NL, NP = 8, 3, 4
P8 = 2 * NP              # 8 sampling points per (head, level)
C, D = 256, 32
W_ = [w for h, w in SHAPES]
H_ = [h for h, w in SHAPES]
LVL_START = [0, 16800, 21000]
L = 22050
PAD_L = [w + 2 for w in W_]                       # 170, 86, 44
NENT_L = [-(-(PAD_L[l] + H_[l] * W_[l] + 2) // 256) * 256 for l in range(NL)]
NR_L = [n // 256 for n in NENT_L]                 # 67, 17, 5 rounds
ESZ = 4 * D                                       # 128 bf16 per entry (256B)
HS_L = [n * ESZ for n in NENT_L]                  # head stride in bf16 elems
LQ = 1700
LQP = 1792               # 14 chunks of 128
NCH = LQP // 128
SLOTS = NH * NL * P8     # 192 (h,l,p) combos per query


def build_program(num_cores=8, dbg=False, mode='full'):
    nc = bacc.Bacc("TRN2", target_bir_lowering=False, debug=False,
                   num_devices=num_cores, num_swdge_queues=4)
    value = nc.dram_tensor("value", [L, 2 * C], F32, kind="ExternalInput")
    query = nc.dram_tensor("query", [LQP, C], F32, kind="ExternalInput")
    refp = nc.dram_tensor("refp", [LQP, 4 * NL], F32, kind="ExternalInput")
    consts = nc.dram_tensor("consts", [6 * SLOTS], F32, kind="ExternalInput")
    Wv = nc.dram_tensor("Wv", [2 * C, C], F32, kind="ExternalInput")
    bvr = nc.dram_tensor("bvr", [1, C], F32, kind="ExternalInput")
    Woff = nc.dram_tensor("Woff", [C, SLOTS * 2], F32, kind="ExternalInput")
    boffr = nc.dram_tensor("boffr", [1, SLOTS * 2], F32, kind="ExternalInput")
    Watt = nc.dram_tensor("Watt", [C, 96], F32, kind="ExternalInput")
    battr = nc.dram_tensor("battr", [1, 96], F32, kind="ExternalInput")
    Wout = nc.dram_tensor("Wout", [C, C], F32, kind="ExternalInput")
    boutr = nc.dram_tensor("boutr", [1, C], F32, kind="ExternalInput")
    out = nc.dram_tensor("out", [LQP, C], F32, kind="ExternalOutput")
    m2l = [nc.dram_tensor(f"m2{l}", [NH * HS_L[l]], BF16, kind="Internal")
           for l in range(NL)]

    from contextlib import ExitStack
    with tile.TileContext(nc) as tc:
      with ExitStack() as ctx:
        # ---------------- constant / parameter loads ----------------
        wp = ctx.enter_context(tc.tile_pool(name="wp", bufs=1))
        ident = wp.tile([128, 128], F32)
        make_identity(nc, ident[:])
        wv_t = [wp.tile([128, C], F32, tag=f"wv{k}", name=f"wv{k}") for k in range(4)]
        for k in range(4):
            nc.sync.dma_start(wv_t[k][:], Wv[128 * k:128 * (k + 1), :])
        woff_t = [wp.tile([128, SLOTS * 2], F32, tag=f"woff{k}", name=f"woff{k}") for k in range(2)]
        watt_t = [wp.tile([128, 96], F32, tag=f"watt{k}", name=f"watt{k}") for k in range(2)]
        wout_t = [wp.tile([128, C], F32, tag=f"wout{k}", name=f"wout{k}") for k in range(2)]
        for k in range(2):
            nc.sync.dma_start(woff_t[k][:], Woff[128 * k:128 * (k + 1), :])
            nc.sync.dma_start(watt_t[k][:], Watt[128 * k:128 * (k + 1), :])
            nc.sync.dma_start(wout_t[k][:], Wout[128 * k:128 * (k + 1), :])
        bv_t = wp.tile([1, C], F32)
        boff_t = wp.tile([1, SLOTS * 2], F32)
        batt_t = wp.tile([1, 96], F32)
        bout_t = wp.tile([1, C], F32)
        nc.sync.dma_start(bv_t[:], bvr[:])
        nc.sync.dma_start(boff_t[:], boffr[:])
        nc.sync.dma_start(batt_t[:], battr[:])
        nc.sync.dma_start(bout_t[:], boutr[:])
        ones_t = wp.tile([1, 128], F32)
        nc.gpsimd.memset(ones_t[:], 1.0)
        cst_row = wp.tile([1, 6 * SLOTS], F32)
        nc.sync.dma_start(cst_row[:], consts.ap().unsqueeze(0))
        cst = wp.tile([128, 6 * SLOTS], F32)
        nc.gpsimd.partition_broadcast(cst[:], cst_row[:])
        WT = cst[:, 0 * SLOTS:1 * SLOTS]
        W1 = cst[:, 1 * SLOTS:2 * SLOTS]
        W2 = cst[:, 2 * SLOTS:3 * SLOTS]
        H1 = cst[:, 3 * SLOTS:4 * SLOTS]
        H2 = cst[:, 4 * SLOTS:5 * SLOTS]
        BS = cst[:, 5 * SLOTS:6 * SLOTS]
        # bf16 copies: Wv, bias, ones, extraction matrices
        wvb = [wp.tile([128, C], BF16, tag=f"wvb{k}", name=f"wvb{k}") for k in range(4)]
        for k in range(4):
            nc.vector.tensor_copy(wvb[k][:], wv_t[k][:])
        bvb = wp.tile([1, C], BF16)
        nc.vector.tensor_copy(bvb[:], bv_t[:])
        onesb = wp.tile([1, 128], BF16)
        nc.gpsimd.memset(onesb[:], 1.0)
        # MB[par] = [zeros64 | M_par | zeros64], M_par[p,j] = 1 iff p = 2j+par.
        iv = ident[:].rearrange("p (j t) -> p j t", t=2)
        MB = [wp.tile([128, 192], BF16, tag=f"MB{p}", name=f"MB{p}") for p in range(2)]
        for p in range(2):
            nc.vector.memset(MB[p][:], 0.0)
            nc.vector.tensor_copy(MB[p][:, 64:128], iv[:, :, p])

        # ---------------- pools ----------------
        p1 = ctx.enter_context(tc.tile_pool(name="p1", bufs=3))
        vrp = ctx.enter_context(tc.tile_pool(name="vrp", bufs=8))
        ptd = ctx.enter_context(tc.tile_pool(name="ptd", bufs=2))
        psum = ctx.enter_context(tc.tile_pool(name="psum", bufs=1, space="PSUM"))
        p2 = ctx.enter_context(tc.tile_pool(name="p2", bufs=1))
        pwp = ctx.enter_context(tc.tile_pool(name="pwp", bufs=1))
        pg = ctx.enter_context(tc.tile_pool(name="pg", bufs=5))

        # ---------------- phase 1: per-level M2 build ----------------
        vtiles = {}

        def emit_proj(t):
            # project value rows [128t, 128t+128) -> bf16 v tile in the ring
            p0 = 128 * t
            vt = vrp.tile([128, C], BF16, tag="vring", name=f"vr{t}")
            rlo, rhi = max(0, -p0), min(128, L - p0)
            if rlo > 0 or rhi < 128:
                nc.vector.memset(vt[:], 0.0)
            if rhi > rlo:
                nr = rhi - rlo
                vin = p1.tile([128, 2 * C], F32, tag="vin")
                nc.sync.dma_start(vin[rlo:rhi, :], value[p0 + rlo:p0 + rhi, :])
                vT = p1.tile([128, 2 * C], BF16, tag="vT")
                for k in range(4):
                    pt = psum.tile([128, 128], F32, tag="tp", bufs=2, name="ptp")
                    nc.tensor.transpose(pt[:, 0:nr], vin[rlo:rhi, 128 * k:128 * (k + 1)],
                                        ident[0:nr, 0:nr])
                    nc.scalar.copy(vT[:, 128 * k:128 * k + nr], pt[:, 0:nr])
                ps = psum.tile([128, SLOTS * 2], F32, tag="mm", bufs=2, name="pmm")
                for k in range(4):
                    nc.tensor.matmul(ps[rlo:rhi, 0:C], vT[:, 128 * k:128 * k + nr],
                                     wvb[k][:], start=(k == 0), stop=False)
                nc.tensor.matmul(ps[rlo:rhi, 0:C], onesb[:, 0:nr], bvb[:],
                                 start=False, stop=True)
                nc.scalar.copy(vt[rlo:rhi, :], ps[rlo:rhi, 0:C])
            vtiles[t] = vt

        def emit_level_build(l):
            W, PADl, LS = W_[l], PAD_L[l], LVL_START[l]
            vtiles.clear()
            nxt = (LS - PADl) // 128
            # quad-corner extraction shifts: P0,P1,P2 then Q0,Q1,Q2 (=P+W)
            shifts = (0, 1, 2, W, W + 1, W + 2)
            for r in range(NR_L[l]):
                e0 = 256 * r
                need_hi = (LS - PADl + e0 + 255 + W + 2 + 1) // 128
                while nxt <= need_hi:
                    emit_proj(nxt)
                    nxt += 1
                # es cols [256*pi : 256*pi+256) = extraction pi
                es = psum.tile([128, 6 * C], F32, tag="es", bufs=1, name="es")
                for pi, sh in enumerate(shifts):
                    base = LS + e0 + sh - PADl
                    for jb in (0, 64):
                        P0 = base + 2 * jb
                        t = P0 // 128
                        s0 = P0 - 128 * t
                        colA, par2 = s0 // 2, s0 % 2
                        dst = es[jb:jb + 64, 256 * pi:256 * pi + 256]
                        if colA == 0:
                            nc.tensor.matmul(dst, MB[par2][:, 64:128], vtiles[t][:],
                                             start=True, stop=True)
                        else:
                            nc.tensor.matmul(dst, MB[par2][:, 64 + colA:128 + colA],
                                             vtiles[t][:], start=True, stop=False)
                            nc.tensor.matmul(dst, MB[par2][:, colA:colA + 64],
                                             vtiles[t + 1][:], start=False, stop=True)
                # assemble: partition j -> entries e0+2j, e0+2j+1, each
                # [P(e), P(e+1), Q(e), Q(e+1)]; extraction source per g-slot:
                TD = ptd.tile([128, 2048], BF16, tag="TD")
                TDv = TD[:].rearrange("p (h g d) -> p h g d", g=8, d=D)
                for gi, pi in enumerate((0, 1, 3, 4, 1, 2, 4, 5)):
                    nc.scalar.copy(
                        TDv[:, :, gi, :],
                        es[:, 256 * pi:256 * pi + 256].rearrange("p (h d) -> p h d", d=D))
                for h in range(NH):
                    seg = m2l[l].ap()[h * HS_L[l] + e0 * ESZ:
                                      h * HS_L[l] + (e0 + 256) * ESZ]
                    nc.sync.dma_start(seg.rearrange("(p c) -> p c", c=256),
                                      TD[:, 256 * h:256 * (h + 1)])

        # ---------------- phase 2 pre-work ----------------
        # A: matmul/softmax part (PE+ACT heavy) -- emitted early.
        # B: coords/coefs/idx part (DVE only) -- emitted inside the l=2 pass.
        rp_t, off_t, aw_t, wrp_t, coefx_t, O_t = {}, {}, {}, {}, {}, {}

        def emit_prework_a(ch):
            q0 = ch * 128
            qin = p2.tile([128, C], F32, tag="qin", bufs=2)
            nc.sync.dma_start(qin[:], query[q0:q0 + 128, :])
            rp = pwp.tile([128, 4 * NL], F32, tag=f"rp{ch}", name=f"rp{ch}")
            nc.sync.dma_start(rp[:], refp[q0:q0 + 128, :])
            qT = p2.tile([128, 256], F32, tag="qT", bufs=2)
            for k in range(2):
                pt2 = psum.tile([128, 128], F32, tag="tp", bufs=2, name="pt2")
                nc.tensor.transpose(pt2[:], qin[:, 128 * k:128 * (k + 1)], ident[:])
                nc.scalar.copy(qT[:, 128 * k:128 * (k + 1)], pt2[:])
            pso = psum.tile([128, SLOTS * 2], F32, tag="mm", bufs=2, name="pso")
            for k in range(2):
                nc.tensor.matmul(pso[:], qT[:, 128 * k:128 * (k + 1)], woff_t[k][:],
                                 start=(k == 0), stop=False)
            nc.tensor.matmul(pso[:], ones_t[:], boff_t[:], start=False, stop=True)
            off = pwp.tile([128, SLOTS * 2], F32, tag=f"off{ch}", name=f"off{ch}")
            nc.scalar.copy(off[:], pso[:])
            psa = psum.tile([128, SLOTS * 2], F32, tag="mm", bufs=2, name="psa")
            for k in range(2):
                nc.tensor.matmul(psa[:, 0:96], qT[:, 128 * k:128 * (k + 1)], watt_t[k][:],
                                 start=(k == 0), stop=False)
            nc.tensor.matmul(psa[:, 0:96], ones_t[:], batt_t[:], start=False, stop=True)
            att = p2.tile([128, 96], F32, tag="att", bufs=2)
            nc.scalar.copy(att[:], psa[:, 0:96])
            rmax = p2.tile([128, 8], F32, tag="rmax")
            nc.vector.tensor_reduce(rmax[:], att[:].rearrange("q (h l) -> q h l", l=12), AX.X, AluOpType.max)
            nc.vector.tensor_tensor(att[:].rearrange("q (h l) -> q h l", l=12),
                                    att[:].rearrange("q (h l) -> q h l", l=12),
                                    rmax[:].unsqueeze(2).broadcast_to((128, 8, 12)), AluOpType.subtract)
            nc.scalar.activation(att[:], att[:], AF.Exp)
            rsum = p2.tile([128, 8], F32, tag="rsum")
            nc.vector.tensor_reduce(rsum[:], att[:].rearrange("q (h l) -> q h l", l=12), AX.X, AluOpType.add)
            nc.vector.reciprocal(rsum[:], rsum[:])
            aw = pwp.tile([128, 96], F32, tag=f"aw{ch}", name=f"aw{ch}")
            nc.vector.tensor_tensor(aw[:].rearrange("q (h l) -> q h l", l=12),
                                    att[:].rearrange("q (h l) -> q h l", l=12),
                                    rsum[:].unsqueeze(2).broadcast_to((128, 8, 12)), AluOpType.mult)
            Ot = pwp.tile([128, C], F32, tag=f"O{ch}", name=f"O{ch}")
            rp_t[ch], off_t[ch], aw_t[ch], O_t[ch] = rp, off, aw, Ot

        def emit_prework_b(ch):
            rp, off, aw = rp_t[ch], off_t[ch], aw_t[ch]
            X = p2.tile([128, SLOTS], F32, tag="X")
            Y = p2.tile([128, SLOTS], F32, tag="Y")
            for du in range(2):
                for xy in range(2):
                    T = (X if xy == 0 else Y)
                    for li in range(NL):
                        dst = T[:].rearrange("q (hl p) -> q hl p", p=P8)[:, li::NL, du * NP:(du + 1) * NP]
                        src0 = off[:].rearrange("q (hl pc) -> q hl pc", pc=16)[:, li::NL, 2 * du + xy:2 * du + xy + 13:4]
                        src1 = rp[:, 4 * li + 2 * du + xy].unsqueeze(1).unsqueeze(2).broadcast_to((128, NH, NP))
                        nc.vector.scalar_tensor_tensor(dst, src0, -0.5, src1, AluOpType.add, AluOpType.add)
            TX = p2.tile([128, SLOTS], F32, tag="TX")
            TY = p2.tile([128, SLOTS], F32, tag="TY")
            X0 = p2.tile([128, SLOTS], F32, tag="X0")
            Y0 = p2.tile([128, SLOTS], F32, tag="Y0")
            MAGIC = 12582912.0  # 1.5 * 2^23: (x+M)-M = round-to-nearest(x)
            nc.vector.tensor_scalar(TX[:], X[:], MAGIC, MAGIC, AluOpType.add, AluOpType.subtract)
            nc.vector.tensor_scalar(TY[:], Y[:], MAGIC, MAGIC, AluOpType.add, AluOpType.subtract)
            nc.vector.tensor_tensor(X0[:], TX[:], X[:], AluOpType.is_gt)
            nc.vector.tensor_tensor(Y0[:], TY[:], Y[:], AluOpType.is_gt)
            nc.vector.tensor_tensor(X0[:], TX[:], X0[:], AluOpType.subtract)  # floor(x)
            nc.vector.tensor_tensor(Y0[:], TY[:], Y0[:], AluOpType.subtract)
            nc.vector.tensor_tensor(TX[:], X[:], X0[:], AluOpType.subtract)   # frac
            nc.vector.tensor_tensor(TY[:], Y[:], Y0[:], AluOpType.subtract)
            UX = p2.tile([128, SLOTS], F32, tag="UX")
            UY = p2.tile([128, SLOTS], F32, tag="UY")
            nc.vector.tensor_tensor(UX[:], W1, X0[:], AluOpType.subtract)   # W-1-x0
            nc.vector.tensor_tensor(UY[:], H1, Y0[:], AluOpType.subtract)
            MX0 = p2.tile([128, SLOTS], F32, tag="MX0")
            MY0 = p2.tile([128, SLOTS], F32, tag="MY0")
            MX1 = p2.tile([128, SLOTS], F32, tag="MX1")
            MY1 = p2.tile([128, SLOTS], F32, tag="MY1")
            nc.vector.tensor_tensor(MX0[:], X0[:], UX[:], AluOpType.min)
            nc.vector.tensor_tensor(MY0[:], Y0[:], UY[:], AluOpType.min)
            UX2 = p2.tile([128, SLOTS], F32, tag="UX2")
            UY2 = p2.tile([128, SLOTS], F32, tag="UY2")
            nc.vector.tensor_tensor(UX2[:], W2, X0[:], AluOpType.subtract)
            nc.vector.tensor_tensor(UY2[:], H2, Y0[:], AluOpType.subtract)
            nc.vector.scalar_tensor_tensor(MX1[:], X0[:], 1.0, UX2[:], AluOpType.add, AluOpType.min)
            nc.vector.scalar_tensor_tensor(MY1[:], Y0[:], 1.0, UY2[:], AluOpType.add, AluOpType.min)
            awsx = p2.tile([128, SLOTS], F32, tag="awsx")
            axv = awsx[:].rearrange("q (hl dp) -> q hl dp", dp=P8)
            avv = aw[:].rearrange("q (hl p) -> q hl p", p=NP)
            nc.vector.tensor_copy(axv[:, :, 0:NP], avv)
            nc.vector.tensor_copy(axv[:, :, NP:P8], avv)
            A = p2.tile([128, SLOTS], F32, tag="A")    # 1-tx
            B = p2.tile([128, SLOTS], F32, tag="B")    # 1-ty
            nc.vector.tensor_scalar(A[:], TX[:], -1.0, 1.0, AluOpType.mult, AluOpType.add)
            nc.vector.tensor_scalar(B[:], TY[:], -1.0, 1.0, AluOpType.mult, AluOpType.add)
            coefx = pwp.tile([128, SLOTS * 4], BF16, tag=f"coefx{ch}", name=f"coefx{ch}")
            cxv = coefx[:].rearrange("q (s c) -> q s c", c=4)
            vv = p2.tile([128, SLOTS], F32, tag="vv")
            wgt = p2.tile([128, SLOTS], F32, tag="wgt")
            # corner order matches quad entry: TL, TR, BL, BR
            for (ci, mx, my, wa, wb) in ((0, MX0, MY0, A, B), (1, MX1, MY0, TX, B),
                                         (2, MX0, MY1, A, TY), (3, MX1, MY1, TX, TY)):
                nc.vector.tensor_tensor(vv[:], mx[:], my[:], AluOpType.min)
                nc.vector.scalar_tensor_tensor(vv[:], vv[:], 0.0, awsx[:], AluOpType.is_ge, AluOpType.mult)
                nc.vector.tensor_tensor(wgt[:], wa[:], wb[:], AluOpType.mult)
                nc.vector.tensor_tensor(cxv[:, :, ci], wgt[:], vv[:], AluOpType.mult)
            X0C = p2.tile([128, SLOTS], F32, tag="X0C")
            Y0C = p2.tile([128, SLOTS], F32, tag="Y0C")
            nc.vector.tensor_scalar(X0C[:], X0[:], -1.0, None, AluOpType.max)
            nc.vector.tensor_tensor(X0C[:], X0C[:], W1, AluOpType.min)
            nc.vector.tensor_scalar(Y0C[:], Y0[:], -1.0, None, AluOpType.max)
            nc.vector.tensor_tensor(Y0C[:], Y0C[:], H1, AluOpType.min)
            IDXF = p2.tile([128, SLOTS], F32, tag="IDXF")
            nc.vector.tensor_tensor(IDXF[:], Y0C[:], WT, AluOpType.mult)
            nc.vector.tensor_tensor(IDXF[:], IDXF[:], X0C[:], AluOpType.add)
            nc.vector.tensor_tensor(IDXF[:], IDXF[:], BS, AluOpType.add)
            IDX32 = p2.tile([128, SLOTS], I32, tag="IDX32")
            nc.vector.tensor_copy(IDX32[:], IDXF[:])
            IDX16 = p2.tile([128, SLOTS], I16, tag="IDX16")
            nc.vector.tensor_copy(IDX16[:], IDX32[:])
            T16 = p2.tile([128, SLOTS], I16, tag="T16")
            nc.vector.stream_shuffle(T16[:], IDX16[:], [(i + 16) % 32 for i in range(32)])
            stage = p2.tile([128, SLOTS * 8], I16, tag="stage", bufs=2)
            nc.vector.memset(stage[0:32, :], 0)
            sv = stage[:].rearrange("p (s j) -> p s j", j=8)
            for k in range(4):
                nc.vector.tensor_copy(sv[0:16, :, 2 * k], IDX16[32 * k:32 * k + 16, :])
                nc.vector.tensor_copy(sv[0:16, :, 2 * k + 1], T16[32 * k:32 * k + 16, :])
            nc.vector.tensor_copy(stage[32:64, :], stage[0:32, :])
            nc.vector.tensor_copy(stage[64:96, :], stage[0:32, :])
            nc.vector.tensor_copy(stage[96:128, :], stage[0:32, :])
            wrp = pwp.tile([128, SLOTS * 8], I16, tag=f"wrp{ch}", name=f"wrp{ch}")
            nc.vector.stream_shuffle(wrp[:], stage[:], [i % 16 for i in range(32)])
            wrp_t[ch], coefx_t[ch] = wrp, coefx

        def emit_unit(l, ch, h):
            s0 = (h * NL + l) * P8        # first slot of (h,l) group
            G = pg.tile([128, P8 * ESZ], BF16, tag="G")
            m2ap = m2l[l].ap()
            m2ap.ap = bass_rust.VecI64Pair([[ESZ, NENT_L[l]], [1, ESZ]])
            m2ap.offset = h * HS_L[l]
            if mode == 'nog':
                nc.vector.memset(G[:], 0.01)
            else:
                nc.gpsimd.dma_gather(
                    G[:].rearrange("q (s e) -> q s e", e=ESZ), m2ap,
                    wrp_t[ch][:, 8 * s0:8 * s0 + 64], P8 * 128, P8 * 128, ESZ,
                    elem_step=ESZ, queue_num=(h * NL + l) % 4,
                    single_packet=False)
            TMP = pg.tile([128, P8 * ESZ], BF16, tag="TMP")
            cb = coefx_t[ch][:, 4 * s0:4 * s0 + 32].unsqueeze(2).broadcast_to((128, 32, 32))
            nc.vector.tensor_tensor(TMP[:].rearrange("q (sc c) -> q sc c", c=32),
                                    G[:].rearrange("q (sc c) -> q sc c", c=32),
                                    cb, AluOpType.mult)
            if l == 2:
                nc.vector.tensor_reduce(O_t[ch][:, D * h:D * (h + 1)],
                                        TMP[:].rearrange("q (sc c) -> q c sc", c=32),
                                        AX.X, AluOpType.add)
            else:
                Or = pg.tile([128, D], F32, tag="Or")
                nc.vector.tensor_reduce(Or[:],
                                        TMP[:].rearrange("q (sc c) -> q c sc", c=32),
                                        AX.X, AluOpType.add)
                nc.vector.tensor_tensor(O_t[ch][:, D * h:D * (h + 1)],
                                        O_t[ch][:, D * h:D * (h + 1)], Or[:],
                                        AluOpType.add)

        # ---------------- emission ----------------
        emit_level_build(2)
        for ch in range(NCH):
            emit_prework_a(ch)
        emit_level_build(1)
        emit_level_build(0)
        for ch in range(NCH):
            emit_prework_b(ch)
            for h in range(NH):
                emit_unit(2, ch, h)
        for l in (1, 0):
            for ch in range(NCH):
                for h in range(NH):
                    emit_unit(l, ch, h)

        # ---------------- out = O @ Wout + bout ----------------
        for ch in range(NCH):
            q0 = ch * 128
            OT = p2.tile([128, 256], F32, tag="OT", bufs=2)
            for k in range(2):
                pt3 = psum.tile([128, 128], F32, tag="tp", bufs=2, name="pt3")
                nc.tensor.transpose(pt3[:], O_t[ch][:, 128 * k:128 * (k + 1)], ident[:])
                nc.scalar.copy(OT[:, 128 * k:128 * (k + 1)], pt3[:])
            pso2 = psum.tile([128, SLOTS * 2], F32, tag="mm", bufs=2, name="pso2")
            for k in range(2):
                nc.tensor.matmul(pso2[:, 0:C], OT[:, 128 * k:128 * (k + 1)], wout_t[k][:],
                                 start=(k == 0), stop=False)
            nc.tensor.matmul(pso2[:, 0:C], ones_t[:], bout_t[:], start=False, stop=True)
            OO = p2.tile([128, C], F32, tag="OO", bufs=2)
            nc.scalar.copy(OO[:], pso2[:, 0:C])
            nc.sync.dma_start(out[q0:q0 + 128, :], OO[:])

    nc.finalize()
    return nc


# ---------------- host-side wrapper ----------------
def prep_core_inputs(inputs, b):
    q = np.zeros((LQP, C), np.float32)
    q[:LQ] = inputs["query"][b]
    rl = inputs["ref_l"][b].transpose(0, 2, 1, 3).reshape(LQ, NL, 2)
    rr = inputs["ref_r"][b].transpose(0, 2, 1, 3).reshape(LQ, NL, 2)
    norm = np.array([[w, h] for h, w in SHAPES], np.float32)
    rp = np.zeros((LQP, NL, 4), np.float32)
    rp[:LQ, :, 0:2] = rl * norm
    rp[:LQ, :, 2:4] = rr * norm
    slot_l = np.repeat(np.tile(np.arange(NL), NH), P8).astype(np.int32)  # slot -> level
    Wl = np.array(W_, np.float32)[slot_l]
    Hl = np.array(H_, np.float32)[slot_l]
    Bs = np.array(PAD_L, np.float32)[slot_l]       # per-level local base
    consts = np.concatenate([Wl, Wl - 1, Wl - 2, Hl - 1, Hl - 2, Bs]).astype(np.float32)
    return {
        "value": np.ascontiguousarray(inputs["value"][b]),
        "query": q,
        "refp": rp.reshape(LQP, 4 * NL).astype(np.float32),
        "consts": consts,
        "Wv": inputs["Wv"], "bvr": inputs["bv"][None, :],
        "Woff": inputs["Woff"], "boffr": inputs["boff"][None, :],
        "Watt": inputs["Watt"], "battr": inputs["batt"][None, :],
        "Wout": inputs["Wout"], "boutr": inputs["bout"][None, :],
    }


LAST_EXEC_NS = None


def kernel(**inputs):
    global LAST_EXEC_NS
    import os
    from concourse.bass_utils import run_bass_kernel_spmd
    nc = build_program(num_cores=8)
    in_maps = [prep_core_inputs(inputs, b) for b in range(8)]
    trace = bool(int(os.environ.get("DKA_TRACE", "0")))
    tdir = None
    if trace:
        tdir = "/tmp/dka_trace"
        os.makedirs(tdir, exist_ok=True)
    res = run_bass_kernel_spmd(nc, in_maps, core_ids=list(range(8)), trace=trace,
                               tmpdir=tdir)
    LAST_EXEC_NS = res.exec_time_ns
    return np.stack([res.results[b]["out"][:LQ] for b in range(8)], 0)


# revision 12
# speedup vs baseline: 1.2743x; 1.0300x over previous
# Multi-scale deformable attention kernel for TRN2 (per-core: one batch element).
#
# v3: per-level bf16 quad-corner M2 maps. Entry e of level l, head h =
#   [v_h(p), v_h(p+1), v_h(p+W), v_h(p+W+1)] (4x32 bf16 = 256B), p = LS_l+e-PAD_l
# so ONE 256B gather returns all 4 bilinear corners (order TL,TR,BL,BR).
# Build is fold-2: partition j holds entries 2j,2j+1 -> 512B-contiguous
# M2 writes at full DMA bandwidth. Levels built smallest-first; gather
# passes run per level (2,1,0) so the level-0 build overlaps the level-2/1
# gathers. Pool-engine descriptor generation (~5ns/idx) is the critical
# resource; the per-chunk coord/coef/idx vector work is emitted inside the
# level-2 pass so gather-buffer recycling never queues behind it.
import sys

sys.path.insert(0, "/opt/trn_rl_repo")
import numpy as np

import concourse.bacc as bacc
import concourse.bass as bass
import concourse.mybir as mybir
import concourse.tile as tile
import bass_rust
from concourse.alu_op_type import AluOpType
from concourse.masks import make_identity

F32 = mybir.dt.float32
BF16 = mybir.dt.bfloat16
I32 = mybir.dt.int32
I16 = mybir.dt.int16
AX = mybir.AxisListType
AF = mybir.ActivationFunctionType

SHAPES = ((100, 168), (50, 84), (25, 42))
NH, NL, NP = 8, 3, 4
P8 = 2 * NP              # 8 sampling points per (head, level)
C, D = 256, 32
W_ = [w for h, w in SHAPES]
H_ = [h for h, w in SHAPES]
LVL_START = [0, 16800, 21000]
L = 22050
PAD_L = [w + 2 for w in W_]                       # 170, 86, 44
NENT_L = [-(-(PAD_L[l] + H_[l] * W_[l] + 2) // 256) * 256 for l in range(NL)]
NR_L = [n // 256 for n in NENT_L]                 # 67, 17, 5 rounds
ESZ = 4 * D                                       # 128 bf16 per entry (256B)
HS_L = [n * ESZ for n in NENT_L]                  # head stride in bf16 elems
LQ = 1700
LQP = 1792               # 14 chunks of 128
NCH = LQP // 128
SLOTS = NH * NL * P8     # 192 (h,l,p) combos per query


def build_program(num_cores=8, dbg=False, mode='full'):
    nc = bacc.Bacc("TRN2", target_bir_lowering=False, debug=False,
                   num_devices=num_cores, num_swdge_queues=4)
    value = nc.dram_tensor("value", [L, 2 * C], F32, kind="ExternalInput")
    query = nc.dram_tensor("query", [LQP, C], F32, kind="ExternalInput")
    refp = nc.dram_tensor("refp", [LQP, 4 * NL], F32, kind="ExternalInput")
    consts = nc.dram_tensor("consts", [6 * SLOTS], F32, kind="ExternalInput")
    Wv = nc.dram_tensor("Wv", [2 * C, C], F32, kind="ExternalInput")
    bvr = nc.dram_tensor("bvr", [1, C], F32, kind="ExternalInput")
    Woff = nc.dram_tensor("Woff", [C, SLOTS * 2], F32, kind="ExternalInput")
    boffr = nc.dram_tensor("boffr", [1, SLOTS * 2], F32, kind="ExternalInput")
    Watt = nc.dram_tensor("Watt", [C, 96], F32, kind="ExternalInput")
    battr = nc.dram_tensor("battr", [1, 96], F32, kind="ExternalInput")
    Wout = nc.dram_tensor("Wout", [C, C], F32, kind="ExternalInput")
    boutr = nc.dram_tensor("boutr", [1, C], F32, kind="ExternalInput")
    out = nc.dram_tensor("out", [LQP, C], F32, kind="ExternalOutput")
    m2l = [nc.dram_tensor(f"m2{l}", [NH * HS_L[l]], BF16, kind="Internal")
           for l in range(NL)]

    from contextlib import ExitStack
    with tile.TileContext(nc) as tc:
      with ExitStack() as ctx:
        # ---------------- constant / parameter loads ----------------
        wp = ctx.enter_context(tc.tile_pool(name="wp", bufs=1))
        ident = wp.tile([128, 128], F32)
        make_identity(nc, ident[:])
        wv_t = [wp.tile([128, C], F32, tag=f"wv{k}", name=f"wv{k}") for k in range(4)]
        for k in range(4):
            nc.sync.dma_start(wv_t[k][:], Wv[128 * k:128 * (k + 1), :])
        woff_t = [wp.tile([128, SLOTS * 2], F32, tag=f"woff{k}", name=f"woff{k}") for k in range(2)]
        watt_t = [wp.tile([128, 96], F32, tag=f"watt{k}", name=f"watt{k}") for k in range(2)]
        wout_t = [wp.tile([128, C], F32, tag=f"wout{k}", name=f"wout{k}") for k in range(2)]
        for k in range(2):
            nc.sync.dma_start(woff_t[k][:], Woff[128 * k:128 * (k + 1), :])
            nc.sync.dma_start(watt_t[k][:], Watt[128 * k:128 * (k + 1), :])
            nc.sync.dma_start(wout_t[k][:], Wout[128 * k:128 * (k + 1), :])
        bv_t = wp.tile([1, C], F32)
        boff_t = wp.tile([1, SLOTS * 2], F32)
        batt_t = wp.tile([1, 96], F32)
        bout_t = wp.tile([1, C], F32)
        nc.sync.dma_start(bv_t[:], bvr[:])
        nc.sync.dma_start(boff_t[:], boffr[:])
        nc.sync.dma_start(batt_t[:], battr[:])
        nc.sync.dma_start(bout_t[:], boutr[:])
        ones_t = wp.tile([1, 128], F32)
        nc.gpsimd.memset(ones_t[:], 1.0)
        cst_row = wp.tile([1, 6 * SLOTS], F32)
        nc.sync.dma_start(cst_row[:], consts.ap().unsqueeze(0))
        cst = wp.tile([128, 6 * SLOTS], F32)
        nc.gpsimd.partition_broadcast(cst[:], cst_row[:])
        WT = cst[:, 0 * SLOTS:1 * SLOTS]
        W1 = cst[:, 1 * SLOTS:2 * SLOTS]
        W2 = cst[:, 2 * SLOTS:3 * SLOTS]
        H1 = cst[:, 3 * SLOTS:4 * SLOTS]
        H2 = cst[:, 4 * SLOTS:5 * SLOTS]
        BS = cst[:, 5 * SLOTS:6 * SLOTS]
        # bf16 copies: Wv, bias, ones, extraction matrices
        wvb = [wp.tile([128, C], BF16, tag=f"wvb{k}", name=f"wvb{k}") for k in range(4)]
        for k in range(4):
            nc.vector.tensor_copy(wvb[k][:], wv_t[k][:])
        bvb = wp.tile([1, C], BF16)
        nc.vector.tensor_copy(bvb[:], bv_t[:])
        onesb = wp.tile([1, 128], BF16)
        nc.gpsimd.memset(onesb[:], 1.0)
        # MB[par] = [zeros64 | M_par | zeros64], M_par[p,j] = 1 iff p = 2j+par.
        iv = ident[:].rearrange("p (j t) -> p j t", t=2)
        MB = [wp.tile([128, 192], BF16, tag=f"MB{p}", name=f"MB{p}") for p in range(2)]
        for p in range(2):
            nc.vector.memset(MB[p][:], 0.0)
            nc.vector.tensor_copy(MB[p][:, 64:128], iv[:, :, p])

        # ---------------- pools ----------------
        p1 = ctx.enter_context(tc.tile_pool(name="p1", bufs=3))
        vrp = ctx.enter_context(tc.tile_pool(name="vrp", bufs=8))
        ptd = ctx.enter_context(tc.tile_pool(name="ptd", bufs=2))
        psum = ctx.enter_context(tc.tile_pool(name="psum", bufs=1, space="PSUM"))
        p2 = ctx.enter_context(tc.tile_pool(name="p2", bufs=1))
        pwp = ctx.enter_context(tc.tile_pool(name="pwp", bufs=1))
        pg = ctx.enter_context(tc.tile_pool(name="pg", bufs=5))

        # ---------------- phase 1: per-level M2 build ----------------
        vtiles = {}

        def emit_proj(t):
            # project value rows [128t, 128t+128) -> bf16 v tile in the ring
            p0 = 128 * t
            vt = vrp.tile([128, C], BF16, tag="vring", name=f"vr{t}")
            rlo, rhi = max(0, -p0), min(128, L - p0)
            if rlo > 0 or rhi < 128:
                nc.vector.memset(vt[:], 0.0)
            if rhi > rlo:
                nr = rhi - rlo
                vin = p1.tile([128, 2 * C], F32, tag="vin")
                nc.sync.dma_start(vin[rlo:rhi, :], value[p0 + rlo:p0 + rhi, :])
                vT = p1.tile([128, 2 * C], BF16, tag="vT")
                for k in range(4):
                    pt = psum.tile([128, 128], F32, tag="tp", bufs=2, name="ptp")
                    nc.tensor.transpose(pt[:, 0:nr], vin[rlo:rhi, 128 * k:128 * (k + 1)],
                                        ident[0:nr, 0:nr])
                    nc.scalar.copy(vT[:, 128 * k:128 * k + nr], pt[:, 0:nr])
                ps = psum.tile([128, SLOTS * 2], F32, tag="mm", bufs=2, name="pmm")
                for k in range(4):
                    nc.tensor.matmul(ps[rlo:rhi, 0:C], vT[:, 128 * k:128 * k + nr],
                                     wvb[k][:], start=(k == 0), stop=False)
                nc.tensor.matmul(ps[rlo:rhi, 0:C], onesb[:, 0:nr], bvb[:],
                                 start=False, stop=True)
                nc.scalar.copy(vt[rlo:rhi, :], ps[rlo:rhi, 0:C])
            vtiles[t] = vt

        def emit_level_build(l):
            W, PADl, LS = W_[l], PAD_L[l], LVL_START[l]
            vtiles.clear()
            nxt = (LS - PADl) // 128
            # quad-corner extraction shifts: P0,P1,P2 then Q0,Q1,Q2 (=P+W)
            shifts = (0, 1, 2, W, W + 1, W + 2)
            for r in range(NR_L[l]):
                e0 = 256 * r
                need_hi = (LS - PADl + e0 + 255 + W + 2 + 1) // 128
                while nxt <= need_hi:
                    emit_proj(nxt)
                    nxt += 1
                # es cols [256*pi : 256*pi+256) = extraction pi
                es = psum.tile([128, 6 * C], F32, tag="es", bufs=1, name="es")
                for pi, sh in enumerate(shifts):
                    base = LS + e0 + sh - PADl
                    for jb in (0, 64):
                        P0 = base + 2 * jb
                        t = P0 // 128
                        s0 = P0 - 128 * t
                        colA, par2 = s0 // 2, s0 % 2
                        dst = es[jb:jb + 64, 256 * pi:256 * pi + 256]
                        if colA == 0:
                            nc.tensor.matmul(dst, MB[par2][:, 64:128], vtiles[t][:],
                                             start=True, stop=True)
                        else:
                            nc.tensor.matmul(dst, MB[par2][:, 64 + colA:128 + colA],
                                             vtiles[t][:], start=True, stop=False)
                            nc.tensor.matmul(dst, MB[par2][:, colA:colA + 64],
                                             vtiles[t + 1][:], start=False, stop=True)
                # assemble: partition j -> entries e0+2j, e0+2j+1, each
                # [P(e), P(e+1), Q(e), Q(e+1)]; extraction source per g-slot:
                TD = ptd.tile([128, 2048], BF16, tag="TD")
                TDv = TD[:].rearrange("p (h g d) -> p h g d", g=8, d=D)
                for gi, pi in enumerate((0, 1, 3, 4, 1, 2, 4, 5)):
                    nc.scalar.copy(
                        TDv[:, :, gi, :],
                        es[:, 256 * pi:256 * pi + 256].rearrange("p (h d) -> p h d", d=D))
                for h in range(NH):
                    seg = m2l[l].ap()[h * HS_L[l] + e0 * ESZ:
                                      h * HS_L[l] + (e0 + 256) * ESZ]
                    nc.sync.dma_start(seg.rearrange("(p c) -> p c", c=256),
                                      TD[:, 256 * h:256 * (h + 1)])

        # ---------------- phase 2 pre-work ----------------
        # A: matmul/softmax part (PE+ACT heavy) -- emitted early.
        # B: coords/coefs/idx part (DVE only) -- emitted inside the l=2 pass.
        rp_t, off_t, aw_t, wrp_t, coefx_t, O_t = {}, {}, {}, {}, {}, {}

        def emit_prework_a(ch):
            q0 = ch * 128
            qin = p2.tile([128, C], F32, tag="qin", bufs=2)
            nc.sync.dma_start(qin[:], query[q0:q0 + 128, :])
            rp = pwp.tile([128, 4 * NL], F32, tag=f"rp{ch}", name=f"rp{ch}")
            nc.sync.dma_start(rp[:], refp[q0:q0 + 128, :])
            qT = p2.tile([128, 256], F32, tag="qT", bufs=2)
            for k in range(2):
                pt2 = psum.tile([128, 128], F32, tag="tp", bufs=2, name="pt2")
                nc.tensor.transpose(pt2[:], qin[:, 128 * k:128 * (k + 1)], ident[:])
                nc.scalar.copy(qT[:, 128 * k:128 * (k + 1)], pt2[:])
            pso = psum.tile([128, SLOTS * 2], F32, tag="mm", bufs=2, name="pso")
            for k in range(2):
                nc.tensor.matmul(pso[:], qT[:, 128 * k:128 * (k + 1)], woff_t[k][:],
                                 start=(k == 0), stop=False)
            nc.tensor.matmul(pso[:], ones_t[:], boff_t[:], start=False, stop=True)
            off = pwp.tile([128, SLOTS * 2], F32, tag=f"off{ch}", name=f"off{ch}")
            nc.scalar.copy(off[:], pso[:])
            psa = psum.tile([128, SLOTS * 2], F32, tag="mm", bufs=2, name="psa")
            for k in range(2):
                nc.tensor.matmul(psa[:, 0:96], qT[:, 128 * k:128 * (k + 1)], watt_t[k][:],
                                 start=(k == 0), stop=False)
            nc.tensor.matmul(psa[:, 0:96], ones_t[:], batt_t[:], start=False, stop=True)
            att = p2.tile([128, 96], F32, tag="att", bufs=2)
            nc.scalar.copy(att[:], psa[:, 0:96])
            rmax = p2.tile([128, 8], F32, tag="rmax")
            nc.vector.tensor_reduce(rmax[:], att[:].rearrange("q (h l) -> q h l", l=12), AX.X, AluOpType.max)
            nc.vector.tensor_tensor(att[:].rearrange("q (h l) -> q h l", l=12),
                                    att[:].rearrange("q (h l) -> q h l", l=12),
                                    rmax[:].unsqueeze(2).broadcast_to((128, 8, 12)), AluOpType.subtract)
            nc.scalar.activation(att[:], att[:], AF.Exp)
            rsum = p2.tile([128, 8], F32, tag="rsum")
            nc.vector.tensor_reduce(rsum[:], att[:].rearrange("q (h l) -> q h l", l=12), AX.X, AluOpType.add)
            nc.vector.reciprocal(rsum[:], rsum[:])
            aw = pwp.tile([128, 96], F32, tag=f"aw{ch}", name=f"aw{ch}")
            nc.vector.tensor_tensor(aw[:].rearrange("q (h l) -> q h l", l=12),
                                    att[:].rearrange("q (h l) -> q h l", l=12),
                                    rsum[:].unsqueeze(2).broadcast_to((128, 8, 12)), AluOpType.mult)
            Ot = pwp.tile([128, C], F32, tag=f"O{ch}", name=f"O{ch}")
            rp_t[ch], off_t[ch], aw_t[ch], O_t[ch] = rp, off, aw, Ot

        def emit_prework_b(ch):
            rp, off, aw = rp_t[ch], off_t[ch], aw_t[ch]
            X = p2.tile([128, SLOTS], F32, tag="X")
            Y = p2.tile([128, SLOTS], F32, tag="Y")
            for du in range(2):
                for xy in range(2):
                    T = (X if xy == 0 else Y)
                    for li in range(NL):
                        dst = T[:].rearrange("q (hl p) -> q hl p", p=P8)[:, li::NL, du * NP:(du + 1) * NP]
                        src0 = off[:].rearrange("q (hl pc) -> q hl pc", pc=16)[:, li::NL, 2 * du + xy:2 * du + xy + 13:4]
                        src1 = rp[:, 4 * li + 2 * du + xy].unsqueeze(1).unsqueeze(2).broadcast_to((128, NH, NP))
                        nc.vector.scalar_tensor_tensor(dst, src0, -0.5, src1, AluOpType.add, AluOpType.add)
            TX = p2.tile([128, SLOTS], F32, tag="TX")
            TY = p2.tile([128, SLOTS], F32, tag="TY")
            X0 = p2.tile([128, SLOTS], F32, tag="X0")
            Y0 = p2.tile([128, SLOTS], F32, tag="Y0")
            MAGIC = 12582912.0  # 1.5 * 2^23: (x+M)-M = round-to-nearest(x)
            nc.vector.tensor_scalar(TX[:], X[:], MAGIC, MAGIC, AluOpType.add, AluOpType.subtract)
            nc.vector.tensor_scalar(TY[:], Y[:], MAGIC, MAGIC, AluOpType.add, AluOpType.subtract)
            nc.vector.tensor_tensor(X0[:], TX[:], X[:], AluOpType.is_gt)
            nc.vector.tensor_tensor(Y0[:], TY[:], Y[:], AluOpType.is_gt)
            nc.vector.tensor_tensor(X0[:], TX[:], X0[:], AluOpType.subtract)  # floor(x)
            nc.vector.tensor_tensor(Y0[:], TY[:], Y0[:], AluOpType.subtract)
            nc.vector.tensor_tensor(TX[:], X[:], X0[:], AluOpType.subtract)   # frac
            nc.vector.tensor_tensor(TY[:], Y[:], Y0[:], AluOpType.subtract)
            UX = p2.tile([128, SLOTS], F32, tag="UX")
            UY = p2.tile([128, SLOTS], F32, tag="UY")
            nc.vector.tensor_tensor(UX[:], W1, X0[:], AluOpType.subtract)   # W-1-x0
            nc.vector.tensor_tensor(UY[:], H1, Y0[:], AluOpType.subtract)
            MX0 = p2.tile([128, SLOTS], F32, tag="MX0")
            MY0 = p2.tile([128, SLOTS], F32, tag="MY0")
            MX1 = p2.tile([128, SLOTS], F32, tag="MX1")
            MY1 = p2.tile([128, SLOTS], F32, tag="MY1")
            nc.vector.tensor_tensor(MX0[:], X0[:], UX[:], AluOpType.min)
            nc.vector.tensor_tensor(MY0[:], Y0[:], UY[:], AluOpType.min)
            UX2 = p2.tile([128, SLOTS], F32, tag="UX2")
            UY2 = p2.tile([128, SLOTS], F32, tag="UY2")
            nc.vector.tensor_tensor(UX2[:], W2, X0[:], AluOpType.subtract)
            nc.vector.tensor_tensor(UY2[:], H2, Y0[:], AluOpType.subtract)
            nc.vector.scalar_tensor_tensor(MX1[:], X0[:], 1.0, UX2[:], AluOpType.add, AluOpType.min)
            nc.vector.scalar_tensor_tensor(MY1[:], Y0[:], 1.0, UY2[:], AluOpType.add, AluOpType.min)
            awsx = p2.tile([128, SLOTS], F32, tag="awsx")
            axv = awsx[:].rearrange("q (hl dp) -> q hl dp", dp=P8)
            avv = aw[:].rearrange("q (hl p) -> q hl p", p=NP)
            nc.vector.tensor_copy(axv[:, :, 0:NP], avv)
            nc.vector.tensor_copy(axv[:, :, NP:P8], avv)
            A = p2.tile([128, SLOTS], F32, tag="A")    # 1-tx
            B = p2.tile([128, SLOTS], F32, tag="B")    # 1-ty
            nc.vector.tensor_scalar(A[:], TX[:], -1.0, 1.0, AluOpType.mult, AluOpType.add)
            nc.vector.tensor_scalar(B[:], TY[:], -1.0, 1.0, AluOpType.mult, AluOpType.add)
            coefx = pwp.tile([128, SLOTS * 4], BF16, tag=f"coefx{ch}", name=f"coefx{ch}")
            cxv = coefx[:].rearrange("q (s c) -> q s c", c=4)
            vv = p2.tile([128, SLOTS], F32, tag="vv")
            wgt = p2.tile([128, SLOTS], F32, tag="wgt")
            # corner order matches quad entry: TL, TR, BL, BR
            for (ci, mx, my, wa, wb) in ((0, MX0, MY0, A, B), (1, MX1, MY0, TX, B),
                                         (2, MX0, MY1, A, TY), (3, MX1, MY1, TX, TY)):
                nc.vector.tensor_tensor(vv[:], mx[:], my[:], AluOpType.min)
                nc.vector.scalar_tensor_tensor(vv[:], vv[:], 0.0, awsx[:], AluOpType.is_ge, AluOpType.mult)
                nc.vector.tensor_tensor(wgt[:], wa[:], wb[:], AluOpType.mult)
                nc.vector.tensor_tensor(cxv[:, :, ci], wgt[:], vv[:], AluOpType.mult)
            X0C = p2.tile([128, SLOTS], F32, tag="X0C")
            Y0C = p2.tile([128, SLOTS], F32, tag="Y0C")
            nc.vector.tensor_scalar(X0C[:], X0[:], -1.0, None, AluOpType.max)
            nc.vector.tensor_tensor(X0C[:], X0C[:], W1, AluOpType.min)
            nc.vector.tensor_scalar(Y0C[:], Y0[:], -1.0, None, AluOpType.max)
            nc.vector.tensor_tensor(Y0C[:], Y0C[:], H1, AluOpType.min)
            IDXF = p2.tile([128, SLOTS], F32, tag="IDXF")
            nc.vector.tensor_tensor(IDXF[:], Y0C[:], WT, AluOpType.mult)
            nc.vector.tensor_tensor(IDXF[:], IDXF[:], X0C[:], AluOpType.add)
            nc.vector.tensor_tensor(IDXF[:], IDXF[:], BS, AluOpType.add)
            IDX32 = p2.tile([128, SLOTS], I32, tag="IDX32")
            nc.vector.tensor_copy(IDX32[:], IDXF[:])
            IDX16 = p2.tile([128, SLOTS], I16, tag="IDX16")
            nc.vector.tensor_copy(IDX16[:], IDX32[:])
            T16 = p2.tile([128, SLOTS], I16, tag="T16")
            nc.vector.stream_shuffle(T16[:], IDX16[:], [(i + 16) % 32 for i in range(32)])
            stage = p2.tile([128, SLOTS * 8], I16, tag="stage", bufs=2)
            nc.vector.memset(stage[0:32, :], 0)
            sv = stage[:].rearrange("p (s j) -> p s j", j=8)
            for k in range(4):
                nc.vector.tensor_copy(sv[0:16, :, 2 * k], IDX16[32 * k:32 * k + 16, :])
                nc.vector.tensor_copy(sv[0:16, :, 2 * k + 1], T16[32 * k:32 * k + 16, :])
            nc.vector.tensor_copy(stage[32:64, :], stage[0:32, :])
            nc.vector.tensor_copy(stage[64:96, :], stage[0:32, :])
            nc.vector.tensor_copy(stage[96:128, :], stage[0:32, :])
            wrp = pwp.tile([128, SLOTS * 8], I16, tag=f"wrp{ch}", name=f"wrp{ch}")
            nc.vector.stream_shuffle(wrp[:], stage[:], [i % 16 for i in range(32)])
            wrp_t[ch], coefx_t[ch] = wrp, coefx

        def emit_unit(l, ch, h):
            s0 = (h * NL + l) * P8        # first slot of (h,l) group
            G = pg.tile([128, P8 * ESZ], BF16, tag="G")
            m2ap = m2l[l].ap()
            m2ap.ap = bass_rust.VecI64Pair([[ESZ, NENT_L[l]], [1, ESZ]])
            m2ap.offset = h * HS_L[l]
            if mode == 'nog':
                nc.vector.memset(G[:], 0.01)
            else:
                nc.gpsimd.dma_gather(
                    G[:].rearrange("q (s e) -> q s e", e=ESZ), m2ap,
                    wrp_t[ch][:, 8 * s0:8 * s0 + 64], P8 * 128, P8 * 128, ESZ,
                    elem_step=ESZ, queue_num=(h * NL + l) % 4,
                    single_packet=False)
            TMP = pg.tile([128, P8 * ESZ], BF16, tag="TMP")
            cb = coefx_t[ch][:, 4 * s0:4 * s0 + 32].unsqueeze(2).broadcast_to((128, 32, 32))
            nc.vector.tensor_tensor(TMP[:].rearrange("q (sc c) -> q sc c", c=32),
                                    G[:].rearrange("q (sc c) -> q sc c", c=32),
                                    cb, AluOpType.mult)
            if l == 2:
                nc.vector.tensor_reduce(O_t[ch][:, D * h:D * (h + 1)],
                                        TMP[:].rearrange("q (sc c) -> q c sc", c=32),
                                        AX.X, AluOpType.add)
            else:
                Or = pg.tile([128, D], F32, tag="Or")
                nc.vector.tensor_reduce(Or[:],
                                        TMP[:].rearrange("q (sc c) -> q c sc", c=32),
                                        AX.X, AluOpType.add)
                nc.vector.tensor_tensor(O_t[ch][:, D * h:D * (h + 1)],
                                        O_t[ch][:, D * h:D * (h + 1)], Or[:],
                                        AluOpType.add)

        # ---------------- emission ----------------
        emit_level_build(2)
        for ch in range(NCH):
            emit_prework_a(ch)
        emit_level_build(1)
        emit_level_build(0)
        # software-pipeline the DVE-side prework 3 chunks ahead of the
        # gathers so the Pool engine never waits on the coord/idx chain
        LOOK = 3
        for ch in range(min(LOOK, NCH)):
            emit_prework_b(ch)
        for ch in range(NCH):
            for h in range(NH):
                emit_unit(2, ch, h)
            if ch + LOOK < NCH:
                emit_prework_b(ch + LOOK)
        for l in (1, 0):
            for ch in range(NCH):
                for h in range(NH):
                    emit_unit(l, ch, h)

        # ---------------- out = O @ Wout + bout ----------------
        for ch in range(NCH):
            q0 = ch * 128
            OT = p2.tile([128, 256], F32, tag="OT", bufs=2)
            for k in range(2):
                pt3 = psum.tile([128, 128], F32, tag="tp", bufs=2, name="pt3")
                nc.tensor.transpose(pt3[:], O_t[ch][:, 128 * k:128 * (k + 1)], ident[:])
                nc.scalar.copy(OT[:, 128 * k:128 * (k + 1)], pt3[:])
            pso2 = psum.tile([128, SLOTS * 2], F32, tag="mm", bufs=2, name="pso2")
            for k in range(2):
                nc.tensor.matmul(pso2[:, 0:C], OT[:, 128 * k:128 * (k + 1)], wout_t[k][:],
                                 start=(k == 0), stop=False)
            nc.tensor.matmul(pso2[:, 0:C], ones_t[:], bout_t[:], start=False, stop=True)
            OO = p2.tile([128, C], F32, tag="OO", bufs=2)
            nc.scalar.copy(OO[:], pso2[:, 0:C])
            nc.sync.dma_start(out[q0:q0 + 128, :], OO[:])

    nc.finalize()
    return nc


# ---------------- host-side wrapper ----------------
def prep_core_inputs(inputs, b):
    q = np.zeros((LQP, C), np.float32)
    q[:LQ] = inputs["query"][b]
    rl = inputs["ref_l"][b].transpose(0, 2, 1, 3).reshape(LQ, NL, 2)
    rr = inputs["ref_r"][b].transpose(0, 2, 1, 3).reshape(LQ, NL, 2)
    norm = np.array([[w, h] for h, w in SHAPES], np.float32)
    rp = np.zeros((LQP, NL, 4), np.float32)
    rp[:LQ, :, 0:2] = rl * norm
    rp[:LQ, :, 2:4] = rr * norm
    slot_l = np.repeat(np.tile(np.arange(NL), NH), P8).astype(np.int32)  # slot -> level
    Wl = np.array(W_, np.float32)[slot_l]
    Hl = np.array(H_, np.float32)[slot_l]
    Bs = np.array(PAD_L, np.float32)[slot_l]       # per-level local base
    consts = np.concatenate([Wl, Wl - 1, Wl - 2, Hl - 1, Hl - 2, Bs]).astype(np.float32)
    return {
        "value": np.ascontiguousarray(inputs["value"][b]),
        "query": q,
        "refp": rp.reshape(LQP, 4 * NL).astype(np.float32),
        "consts": consts,
        "Wv": inputs["Wv"], "bvr": inputs["bv"][None, :],
        "Woff": inputs["Woff"], "boffr": inputs["boff"][None, :],
        "Watt": inputs["Watt"], "battr": inputs["batt"][None, :],
        "Wout": inputs["Wout"], "boutr": inputs["bout"][None, :],
    }


LAST_EXEC_NS = None


def kernel(**inputs):
    global LAST_EXEC_NS
    import os
    from concourse.bass_utils import run_bass_kernel_spmd
    nc = build_program(num_cores=8)
    in_maps = [prep_core_inputs(inputs, b) for b in range(8)]
    trace = bool(int(os.environ.get("DKA_TRACE", "0")))
    tdir = None
    if trace:
        tdir = "/tmp/dka_trace"
        os.makedirs(tdir, exist_ok=True)
    res = run_bass_kernel_spmd(nc, in_maps, core_ids=list(range(8)), trace=trace,
                               tmpdir=tdir)
    LAST_EXEC_NS = res.exec_time_ns
    return np.stack([res.results[b]["out"][:LQ] for b in range(8)], 0)


# revision 14
# speedup vs baseline: 1.2873x; 1.0102x over previous
# Multi-scale deformable attention kernel for TRN2 (per-core: one batch element).
#
# v3: per-level bf16 quad-corner M2 maps. Entry e of level l, head h =
#   [v_h(p), v_h(p+1), v_h(p+W), v_h(p+W+1)] (4x32 bf16 = 256B), p = LS_l+e-PAD_l
# so ONE 256B gather returns all 4 bilinear corners (order TL,TR,BL,BR).
# Build is fold-2: partition j holds entries 2j,2j+1 -> 512B-contiguous
# M2 writes at full DMA bandwidth. Levels built smallest-first; gather
# passes run per level (2,1,0) so the level-0 build overlaps the level-2/1
# gathers. Pool-engine descriptor generation (~5ns/idx) is the critical
# resource; the per-chunk coord/coef/idx vector work is emitted inside the
# level-2 pass so gather-buffer recycling never queues behind it.
import sys

sys.path.insert(0, "/opt/trn_rl_repo")
import numpy as np

import concourse.bacc as bacc
import concourse.bass as bass
import concourse.mybir as mybir
import concourse.tile as tile
import bass_rust
from concourse.alu_op_type import AluOpType
from concourse.masks import make_identity

F32 = mybir.dt.float32
BF16 = mybir.dt.bfloat16
I32 = mybir.dt.int32
I16 = mybir.dt.int16
AX = mybir.AxisListType
AF = mybir.ActivationFunctionType

SHAPES = ((100, 168), (50, 84), (25, 42))
NH, NL, NP = 8, 3, 4
P8 = 2 * NP              # 8 sampling points per (head, level)
C, D = 256, 32
W_ = [w for h, w in SHAPES]
H_ = [h for h, w in SHAPES]
LVL_START = [0, 16800, 21000]
L = 22050
PAD_L = [w + 2 for w in W_]                       # 170, 86, 44
NENT_L = [-(-(PAD_L[l] + H_[l] * W_[l] + 2) // 256) * 256 for l in range(NL)]
NR_L = [n // 256 for n in NENT_L]                 # 67, 17, 5 rounds
ESZ = 4 * D                                       # 128 bf16 per entry (256B)
HS_L = [n * ESZ for n in NENT_L]                  # head stride in bf16 elems
LQ = 1700
LQP = 1792               # 14 chunks of 128
NCH = LQP // 128
SLOTS = NH * NL * P8     # 192 (h,l,p) combos per query


def build_program(num_cores=8, dbg=False, mode='full'):
    nc = bacc.Bacc("TRN2", target_bir_lowering=False, debug=False,
                   num_devices=num_cores, num_swdge_queues=4)
    value = nc.dram_tensor("value", [L, 2 * C], F32, kind="ExternalInput")
    query = nc.dram_tensor("query", [LQP, C], F32, kind="ExternalInput")
    refp = nc.dram_tensor("refp", [LQP, 4 * NL], F32, kind="ExternalInput")
    consts = nc.dram_tensor("consts", [6 * SLOTS], F32, kind="ExternalInput")
    Wv = nc.dram_tensor("Wv", [2 * C, C], F32, kind="ExternalInput")
    bvr = nc.dram_tensor("bvr", [1, C], F32, kind="ExternalInput")
    Woff = nc.dram_tensor("Woff", [C, SLOTS * 2], F32, kind="ExternalInput")
    boffr = nc.dram_tensor("boffr", [1, SLOTS * 2], F32, kind="ExternalInput")
    Watt = nc.dram_tensor("Watt", [C, 96], F32, kind="ExternalInput")
    battr = nc.dram_tensor("battr", [1, 96], F32, kind="ExternalInput")
    Wout = nc.dram_tensor("Wout", [C, C], F32, kind="ExternalInput")
    boutr = nc.dram_tensor("boutr", [1, C], F32, kind="ExternalInput")
    out = nc.dram_tensor("out", [LQP, C], F32, kind="ExternalOutput")
    m2l = [nc.dram_tensor(f"m2{l}", [NH * HS_L[l]], BF16, kind="Internal")
           for l in range(NL)]

    from contextlib import ExitStack
    with tile.TileContext(nc) as tc:
      with ExitStack() as ctx:
        # ---------------- constant / parameter loads ----------------
        wp = ctx.enter_context(tc.tile_pool(name="wp", bufs=1))
        ident = wp.tile([128, 128], F32)
        make_identity(nc, ident[:])
        wv_t = [wp.tile([128, C], F32, tag=f"wv{k}", name=f"wv{k}") for k in range(4)]
        for k in range(4):
            nc.sync.dma_start(wv_t[k][:], Wv[128 * k:128 * (k + 1), :])
        woff_t = [wp.tile([128, SLOTS * 2], F32, tag=f"woff{k}", name=f"woff{k}") for k in range(2)]
        watt_t = [wp.tile([128, 96], F32, tag=f"watt{k}", name=f"watt{k}") for k in range(2)]
        wout_t = [wp.tile([128, C], F32, tag=f"wout{k}", name=f"wout{k}") for k in range(2)]
        for k in range(2):
            nc.sync.dma_start(woff_t[k][:], Woff[128 * k:128 * (k + 1), :])
            nc.sync.dma_start(watt_t[k][:], Watt[128 * k:128 * (k + 1), :])
            nc.sync.dma_start(wout_t[k][:], Wout[128 * k:128 * (k + 1), :])
        bv_t = wp.tile([1, C], F32)
        boff_t = wp.tile([1, SLOTS * 2], F32)
        batt_t = wp.tile([1, 96], F32)
        bout_t = wp.tile([1, C], F32)
        nc.sync.dma_start(bv_t[:], bvr[:])
        nc.sync.dma_start(boff_t[:], boffr[:])
        nc.sync.dma_start(batt_t[:], battr[:])
        nc.sync.dma_start(bout_t[:], boutr[:])
        ones_t = wp.tile([1, 128], F32)
        nc.gpsimd.memset(ones_t[:], 1.0)
        cst_row = wp.tile([1, 6 * SLOTS], F32)
        nc.sync.dma_start(cst_row[:], consts.ap().unsqueeze(0))
        cst = wp.tile([128, 6 * SLOTS], F32)
        nc.gpsimd.partition_broadcast(cst[:], cst_row[:])
        WT = cst[:, 0 * SLOTS:1 * SLOTS]
        W1 = cst[:, 1 * SLOTS:2 * SLOTS]
        W2 = cst[:, 2 * SLOTS:3 * SLOTS]
        H1 = cst[:, 3 * SLOTS:4 * SLOTS]
        H2 = cst[:, 4 * SLOTS:5 * SLOTS]
        BS = cst[:, 5 * SLOTS:6 * SLOTS]
        # bf16 copies: Wv, bias, ones, extraction matrices
        wvb = [wp.tile([128, C], BF16, tag=f"wvb{k}", name=f"wvb{k}") for k in range(4)]
        for k in range(4):
            nc.vector.tensor_copy(wvb[k][:], wv_t[k][:])
        bvb = wp.tile([1, C], BF16)
        nc.vector.tensor_copy(bvb[:], bv_t[:])
        onesb = wp.tile([1, 128], BF16)
        nc.gpsimd.memset(onesb[:], 1.0)
        # MB[par] = [zeros64 | M_par | zeros64], M_par[p,j] = 1 iff p = 2j+par.
        iv = ident[:].rearrange("p (j t) -> p j t", t=2)
        MB = [wp.tile([128, 192], BF16, tag=f"MB{p}", name=f"MB{p}") for p in range(2)]
        for p in range(2):
            nc.vector.memset(MB[p][:], 0.0)
            nc.vector.tensor_copy(MB[p][:, 64:128], iv[:, :, p])

        # ---------------- pools ----------------
        p1 = ctx.enter_context(tc.tile_pool(name="p1", bufs=3))
        vrp = ctx.enter_context(tc.tile_pool(name="vrp", bufs=8))
        ptd = ctx.enter_context(tc.tile_pool(name="ptd", bufs=2))
        psum = ctx.enter_context(tc.tile_pool(name="psum", bufs=1, space="PSUM"))
        p2 = ctx.enter_context(tc.tile_pool(name="p2", bufs=1))
        pwp = ctx.enter_context(tc.tile_pool(name="pwp", bufs=1))
        pg = ctx.enter_context(tc.tile_pool(name="pg", bufs=5))

        # ---------------- phase 1: per-level M2 build ----------------
        vtiles = {}

        def emit_proj(t):
            # project value rows [128t, 128t+128) -> bf16 v tile in the ring
            p0 = 128 * t
            vt = vrp.tile([128, C], BF16, tag="vring", name=f"vr{t}")
            rlo, rhi = max(0, -p0), min(128, L - p0)
            if rlo > 0 or rhi < 128:
                nc.vector.memset(vt[:], 0.0)
            if rhi > rlo:
                nr = rhi - rlo
                vin = p1.tile([128, 2 * C], F32, tag="vin")
                nc.sync.dma_start(vin[rlo:rhi, :], value[p0 + rlo:p0 + rhi, :])
                vT = p1.tile([128, 2 * C], BF16, tag="vT")
                for k in range(4):
                    pt = psum.tile([128, 128], F32, tag="tp", bufs=2, name="ptp")
                    nc.tensor.transpose(pt[:, 0:nr], vin[rlo:rhi, 128 * k:128 * (k + 1)],
                                        ident[0:nr, 0:nr])
                    nc.scalar.copy(vT[:, 128 * k:128 * k + nr], pt[:, 0:nr])
                ps = psum.tile([128, SLOTS * 2], F32, tag="mm", bufs=2, name="pmm")
                for k in range(4):
                    nc.tensor.matmul(ps[rlo:rhi, 0:C], vT[:, 128 * k:128 * k + nr],
                                     wvb[k][:], start=(k == 0), stop=False)
                nc.tensor.matmul(ps[rlo:rhi, 0:C], onesb[:, 0:nr], bvb[:],
                                 start=False, stop=True)
                nc.scalar.copy(vt[rlo:rhi, :], ps[rlo:rhi, 0:C])
            vtiles[t] = vt

        def emit_level_build(l):
            W, PADl, LS = W_[l], PAD_L[l], LVL_START[l]
            vtiles.clear()
            nxt = (LS - PADl) // 128
            # quad-corner extraction shifts: P0,P1,P2 then Q0,Q1,Q2 (=P+W)
            shifts = (0, 1, 2, W, W + 1, W + 2)
            for r in range(NR_L[l]):
                e0 = 256 * r
                need_hi = (LS - PADl + e0 + 255 + W + 2 + 1) // 128
                while nxt <= need_hi:
                    emit_proj(nxt)
                    nxt += 1
                # es cols [256*pi : 256*pi+256) = extraction pi
                es = psum.tile([128, 6 * C], F32, tag="es", bufs=1, name="es")
                for pi, sh in enumerate(shifts):
                    base = LS + e0 + sh - PADl
                    for jb in (0, 64):
                        P0 = base + 2 * jb
                        t = P0 // 128
                        s0 = P0 - 128 * t
                        colA, par2 = s0 // 2, s0 % 2
                        dst = es[jb:jb + 64, 256 * pi:256 * pi + 256]
                        if colA == 0:
                            nc.tensor.matmul(dst, MB[par2][:, 64:128], vtiles[t][:],
                                             start=True, stop=True)
                        else:
                            nc.tensor.matmul(dst, MB[par2][:, 64 + colA:128 + colA],
                                             vtiles[t][:], start=True, stop=False)
                            nc.tensor.matmul(dst, MB[par2][:, colA:colA + 64],
                                             vtiles[t + 1][:], start=False, stop=True)
                # assemble: partition j -> entries e0+2j, e0+2j+1, each
                # [P(e), P(e+1), Q(e), Q(e+1)]; extraction source per g-slot:
                TD = ptd.tile([128, 2048], BF16, tag="TD")
                TDv = TD[:].rearrange("p (h g d) -> p h g d", g=8, d=D)
                for gi, pi in enumerate((0, 1, 3, 4, 1, 2, 4, 5)):
                    nc.scalar.copy(
                        TDv[:, :, gi, :],
                        es[:, 256 * pi:256 * pi + 256].rearrange("p (h d) -> p h d", d=D))
                for h in range(NH):
                    seg = m2l[l].ap()[h * HS_L[l] + e0 * ESZ:
                                      h * HS_L[l] + (e0 + 256) * ESZ]
                    nc.sync.dma_start(seg.rearrange("(p c) -> p c", c=256),
                                      TD[:, 256 * h:256 * (h + 1)])

        # ---------------- phase 2 pre-work ----------------
        # A: matmul/softmax part (PE+ACT heavy) -- emitted early.
        # B: coords/coefs/idx part (DVE only) -- emitted inside the l=2 pass.
        rp_t, off_t, aw_t, wrp_t, coefx_t, O_t = {}, {}, {}, {}, {}, {}

        def emit_prework_a(ch):
            q0 = ch * 128
            qin = p2.tile([128, C], F32, tag="qin", bufs=2)
            nc.sync.dma_start(qin[:], query[q0:q0 + 128, :])
            rp = pwp.tile([128, 4 * NL], F32, tag=f"rp{ch}", name=f"rp{ch}")
            nc.sync.dma_start(rp[:], refp[q0:q0 + 128, :])
            qT = p2.tile([128, 256], F32, tag="qT", bufs=2)
            for k in range(2):
                pt2 = psum.tile([128, 128], F32, tag="tp", bufs=2, name="pt2")
                nc.tensor.transpose(pt2[:], qin[:, 128 * k:128 * (k + 1)], ident[:])
                nc.scalar.copy(qT[:, 128 * k:128 * (k + 1)], pt2[:])
            pso = psum.tile([128, SLOTS * 2], F32, tag="mm", bufs=2, name="pso")
            for k in range(2):
                nc.tensor.matmul(pso[:], qT[:, 128 * k:128 * (k + 1)], woff_t[k][:],
                                 start=(k == 0), stop=False)
            nc.tensor.matmul(pso[:], ones_t[:], boff_t[:], start=False, stop=True)
            off = pwp.tile([128, SLOTS * 2], F32, tag=f"off{ch}", name=f"off{ch}")
            nc.scalar.copy(off[:], pso[:])
            psa = psum.tile([128, SLOTS * 2], F32, tag="mm", bufs=2, name="psa")
            for k in range(2):
                nc.tensor.matmul(psa[:, 0:96], qT[:, 128 * k:128 * (k + 1)], watt_t[k][:],
                                 start=(k == 0), stop=False)
            nc.tensor.matmul(psa[:, 0:96], ones_t[:], batt_t[:], start=False, stop=True)
            att = p2.tile([128, 96], F32, tag="att", bufs=2)
            nc.scalar.copy(att[:], psa[:, 0:96])
            rmax = p2.tile([128, 8], F32, tag="rmax")
            nc.vector.tensor_reduce(rmax[:], att[:].rearrange("q (h l) -> q h l", l=12), AX.X, AluOpType.max)
            nc.vector.tensor_tensor(att[:].rearrange("q (h l) -> q h l", l=12),
                                    att[:].rearrange("q (h l) -> q h l", l=12),
                                    rmax[:].unsqueeze(2).broadcast_to((128, 8, 12)), AluOpType.subtract)
            nc.scalar.activation(att[:], att[:], AF.Exp)
            rsum = p2.tile([128, 8], F32, tag="rsum")
            nc.vector.tensor_reduce(rsum[:], att[:].rearrange("q (h l) -> q h l", l=12), AX.X, AluOpType.add)
            nc.vector.reciprocal(rsum[:], rsum[:])
            aw = pwp.tile([128, 96], F32, tag=f"aw{ch}", name=f"aw{ch}")
            nc.vector.tensor_tensor(aw[:].rearrange("q (h l) -> q h l", l=12),
                                    att[:].rearrange("q (h l) -> q h l", l=12),
                                    rsum[:].unsqueeze(2).broadcast_to((128, 8, 12)), AluOpType.mult)
            Ot = pwp.tile([128, C], F32, tag=f"O{ch}", name=f"O{ch}")
            rp_t[ch], off_t[ch], aw_t[ch], O_t[ch] = rp, off, aw, Ot

        def emit_prework_b(ch):
            rp, off, aw = rp_t[ch], off_t[ch], aw_t[ch]
            X = p2.tile([128, SLOTS], F32, tag="X")
            Y = p2.tile([128, SLOTS], F32, tag="Y")
            for du in range(2):
                for xy in range(2):
                    T = (X if xy == 0 else Y)
                    for li in range(NL):
                        dst = T[:].rearrange("q (hl p) -> q hl p", p=P8)[:, li::NL, du * NP:(du + 1) * NP]
                        src0 = off[:].rearrange("q (hl pc) -> q hl pc", pc=16)[:, li::NL, 2 * du + xy:2 * du + xy + 13:4]
                        src1 = rp[:, 4 * li + 2 * du + xy].unsqueeze(1).unsqueeze(2).broadcast_to((128, NH, NP))
                        nc.vector.scalar_tensor_tensor(dst, src0, -0.5, src1, AluOpType.add, AluOpType.add)
            TX = p2.tile([128, SLOTS], F32, tag="TX")
            TY = p2.tile([128, SLOTS], F32, tag="TY")
            X0 = p2.tile([128, SLOTS], F32, tag="X0")
            Y0 = p2.tile([128, SLOTS], F32, tag="Y0")
            MAGIC = 12582912.0  # 1.5 * 2^23: (x+M)-M = round-to-nearest(x)
            nc.vector.tensor_scalar(TX[:], X[:], MAGIC, MAGIC, AluOpType.add, AluOpType.subtract)
            nc.vector.tensor_scalar(TY[:], Y[:], MAGIC, MAGIC, AluOpType.add, AluOpType.subtract)
            nc.vector.tensor_tensor(X0[:], TX[:], X[:], AluOpType.is_gt)
            nc.vector.tensor_tensor(Y0[:], TY[:], Y[:], AluOpType.is_gt)
            nc.vector.tensor_tensor(X0[:], TX[:], X0[:], AluOpType.subtract)  # floor(x)
            nc.vector.tensor_tensor(Y0[:], TY[:], Y0[:], AluOpType.subtract)
            nc.vector.tensor_tensor(TX[:], X[:], X0[:], AluOpType.subtract)   # frac
            nc.vector.tensor_tensor(TY[:], Y[:], Y0[:], AluOpType.subtract)
            UX = p2.tile([128, SLOTS], F32, tag="UX")
            UY = p2.tile([128, SLOTS], F32, tag="UY")
            nc.vector.tensor_tensor(UX[:], W1, X0[:], AluOpType.subtract)   # W-1-x0
            nc.vector.tensor_tensor(UY[:], H1, Y0[:], AluOpType.subtract)
            MX0 = p2.tile([128, SLOTS], F32, tag="MX0")
            MY0 = p2.tile([128, SLOTS], F32, tag="MY0")
            MX1 = p2.tile([128, SLOTS], F32, tag="MX1")
            MY1 = p2.tile([128, SLOTS], F32, tag="MY1")
            nc.vector.tensor_tensor(MX0[:], X0[:], UX[:], AluOpType.min)
            nc.vector.tensor_tensor(MY0[:], Y0[:], UY[:], AluOpType.min)
            UX2 = p2.tile([128, SLOTS], F32, tag="UX2")
            UY2 = p2.tile([128, SLOTS], F32, tag="UY2")
            nc.vector.tensor_tensor(UX2[:], W2, X0[:], AluOpType.subtract)
            nc.vector.tensor_tensor(UY2[:], H2, Y0[:], AluOpType.subtract)
            nc.vector.scalar_tensor_tensor(MX1[:], X0[:], 1.0, UX2[:], AluOpType.add, AluOpType.min)
            nc.vector.scalar_tensor_tensor(MY1[:], Y0[:], 1.0, UY2[:], AluOpType.add, AluOpType.min)
            awsx = p2.tile([128, SLOTS], F32, tag="awsx")
            axv = awsx[:].rearrange("q (hl dp) -> q hl dp", dp=P8)
            avv = aw[:].rearrange("q (hl p) -> q hl p", p=NP)
            nc.vector.tensor_copy(axv[:, :, 0:NP], avv)
            nc.vector.tensor_copy(axv[:, :, NP:P8], avv)
            A = p2.tile([128, SLOTS], F32, tag="A")    # 1-tx
            B = p2.tile([128, SLOTS], F32, tag="B")    # 1-ty
            nc.vector.tensor_scalar(A[:], TX[:], -1.0, 1.0, AluOpType.mult, AluOpType.add)
            nc.vector.tensor_scalar(B[:], TY[:], -1.0, 1.0, AluOpType.mult, AluOpType.add)
            coefx = pwp.tile([128, SLOTS * 4], BF16, tag=f"coefx{ch}", name=f"coefx{ch}")
            cxv = coefx[:].rearrange("q (s c) -> q s c", c=4)
            vv = p2.tile([128, SLOTS], F32, tag="vv")
            wgt = p2.tile([128, SLOTS], F32, tag="wgt")
            # corner order matches quad entry: TL, TR, BL, BR
            for (ci, mx, my, wa, wb) in ((0, MX0, MY0, A, B), (1, MX1, MY0, TX, B),
                                         (2, MX0, MY1, A, TY), (3, MX1, MY1, TX, TY)):
                nc.vector.tensor_tensor(vv[:], mx[:], my[:], AluOpType.min)
                nc.vector.scalar_tensor_tensor(vv[:], vv[:], 0.0, awsx[:], AluOpType.is_ge, AluOpType.mult)
                nc.vector.tensor_tensor(wgt[:], wa[:], wb[:], AluOpType.mult)
                nc.vector.tensor_tensor(cxv[:, :, ci], wgt[:], vv[:], AluOpType.mult)
            X0C = p2.tile([128, SLOTS], F32, tag="X0C")
            Y0C = p2.tile([128, SLOTS], F32, tag="Y0C")
            nc.vector.scalar_tensor_tensor(X0C[:], X0[:], -1.0, W1, AluOpType.max, AluOpType.min)
            nc.vector.scalar_tensor_tensor(Y0C[:], Y0[:], -1.0, H1, AluOpType.max, AluOpType.min)
            IDXF = p2.tile([128, SLOTS], F32, tag="IDXF")
            nc.vector.tensor_tensor(IDXF[:], Y0C[:], WT, AluOpType.mult)
            nc.vector.tensor_tensor(IDXF[:], IDXF[:], X0C[:], AluOpType.add)
            nc.vector.tensor_tensor(IDXF[:], IDXF[:], BS, AluOpType.add)
            IDX32 = p2.tile([128, SLOTS], I32, tag="IDX32")
            nc.vector.tensor_copy(IDX32[:], IDXF[:])
            IDX16 = p2.tile([128, SLOTS], I16, tag="IDX16")
            nc.vector.tensor_copy(IDX16[:], IDX32[:])
            T16 = p2.tile([128, SLOTS], I16, tag="T16")
            nc.vector.stream_shuffle(T16[:], IDX16[:], [(i + 16) % 32 for i in range(32)])
            stage = p2.tile([128, SLOTS * 8], I16, tag="stage", bufs=2)
            nc.vector.memset(stage[0:32, :], 0)
            sv = stage[:].rearrange("p (s j) -> p s j", j=8)
            for k in range(4):
                nc.vector.tensor_copy(sv[0:16, :, 2 * k], IDX16[32 * k:32 * k + 16, :])
                nc.vector.tensor_copy(sv[0:16, :, 2 * k + 1], T16[32 * k:32 * k + 16, :])
            nc.vector.tensor_copy(stage[32:64, :], stage[0:32, :])
            nc.vector.tensor_copy(stage[64:96, :], stage[0:32, :])
            nc.vector.tensor_copy(stage[96:128, :], stage[0:32, :])
            wrp = pwp.tile([128, SLOTS * 8], I16, tag=f"wrp{ch}", name=f"wrp{ch}")
            nc.vector.stream_shuffle(wrp[:], stage[:], [i % 16 for i in range(32)])
            wrp_t[ch], coefx_t[ch] = wrp, coefx

        def emit_unit(l, ch, h):
            s0 = (h * NL + l) * P8        # first slot of (h,l) group
            G = pg.tile([128, P8 * ESZ], BF16, tag="G")
            m2ap = m2l[l].ap()
            m2ap.ap = bass_rust.VecI64Pair([[ESZ, NENT_L[l]], [1, ESZ]])
            m2ap.offset = h * HS_L[l]
            if mode == 'nog':
                nc.vector.memset(G[:], 0.01)
            else:
                nc.gpsimd.dma_gather(
                    G[:].rearrange("q (s e) -> q s e", e=ESZ), m2ap,
                    wrp_t[ch][:, 8 * s0:8 * s0 + 64], P8 * 128, P8 * 128, ESZ,
                    elem_step=ESZ, queue_num=(h * NL + l) % 4,
                    single_packet=False)
            TMP = pg.tile([128, P8 * ESZ], BF16, tag="TMP")
            cb = coefx_t[ch][:, 4 * s0:4 * s0 + 32].unsqueeze(2).broadcast_to((128, 32, 32))
            nc.vector.tensor_tensor(TMP[:].rearrange("q (sc c) -> q sc c", c=32),
                                    G[:].rearrange("q (sc c) -> q sc c", c=32),
                                    cb, AluOpType.mult)
            if l == 2:
                nc.vector.tensor_reduce(O_t[ch][:, D * h:D * (h + 1)],
                                        TMP[:].rearrange("q (sc c) -> q c sc", c=32),
                                        AX.X, AluOpType.add)
            else:
                Or = pg.tile([128, D], F32, tag="Or")
                nc.vector.tensor_reduce(Or[:],
                                        TMP[:].rearrange("q (sc c) -> q c sc", c=32),
                                        AX.X, AluOpType.add)
                nc.vector.tensor_tensor(O_t[ch][:, D * h:D * (h + 1)],
                                        O_t[ch][:, D * h:D * (h + 1)], Or[:],
                                        AluOpType.add)

        # ---------------- emission ----------------
        # level-2 map and the first chunks' prework go first so the first
        # gathers issue as early as possible; the big level-0 build overlaps
        # the level-2/1 gather passes.
        emit_level_build(2)
        for ch in range(4):
            emit_prework_a(ch)
        emit_prework_b(0)
        emit_prework_b(1)
        emit_level_build(1)
        for ch in range(4, NCH):
            emit_prework_a(ch)
        emit_level_build(0)
        # software-pipeline the DVE-side prework 3 chunks ahead of the
        # gathers so the Pool engine never waits on the coord/idx chain
        LOOK = 3
        for ch in range(2, min(LOOK, NCH)):
            emit_prework_b(ch)
        for ch in range(NCH):
            for h in range(NH):
                emit_unit(2, ch, h)
            if ch + LOOK < NCH:
                emit_prework_b(ch + LOOK)
        for l in (1, 0):
            for ch in range(NCH):
                for h in range(NH):
                    emit_unit(l, ch, h)

        # ---------------- out = O @ Wout + bout ----------------
        for ch in range(NCH):
            q0 = ch * 128
            OT = p2.tile([128, 256], F32, tag="OT", bufs=2)
            for k in range(2):
                pt3 = psum.tile([128, 128], F32, tag="tp", bufs=2, name="pt3")
                nc.tensor.transpose(pt3[:], O_t[ch][:, 128 * k:128 * (k + 1)], ident[:])
                nc.scalar.copy(OT[:, 128 * k:128 * (k + 1)], pt3[:])
            pso2 = psum.tile([128, SLOTS * 2], F32, tag="mm", bufs=2, name="pso2")
            for k in range(2):
                nc.tensor.matmul(pso2[:, 0:C], OT[:, 128 * k:128 * (k + 1)], wout_t[k][:],
                                 start=(k == 0), stop=False)
            nc.tensor.matmul(pso2[:, 0:C], ones_t[:], bout_t[:], start=False, stop=True)
            OO = p2.tile([128, C], F32, tag="OO", bufs=2)
            nc.scalar.copy(OO[:], pso2[:, 0:C])
            nc.sync.dma_start(out[q0:q0 + 128, :], OO[:])

    nc.finalize()
    return nc


# ---------------- host-side wrapper ----------------
def prep_core_inputs(inputs, b):
    q = np.zeros((LQP, C), np.float32)
    q[:LQ] = inputs["query"][b]
    rl = inputs["ref_l"][b].transpose(0, 2, 1, 3).reshape(LQ, NL, 2)
    rr = inputs["ref_r"][b].transpose(0, 2, 1, 3).reshape(LQ, NL, 2)
    norm = np.array([[w, h] for h, w in SHAPES], np.float32)
    rp = np.zeros((LQP, NL, 4), np.float32)
    rp[:LQ, :, 0:2] = rl * norm
    rp[:LQ, :, 2:4] = rr * norm
    slot_l = np.repeat(np.tile(np.arange(NL), NH), P8).astype(np.int32)  # slot -> level
    Wl = np.array(W_, np.float32)[slot_l]
    Hl = np.array(H_, np.float32)[slot_l]
    Bs = np.array(PAD_L, np.float32)[slot_l]       # per-level local base
    consts = np.concatenate([Wl, Wl - 1, Wl - 2, Hl - 1, Hl - 2, Bs]).astype(np.float32)
    return {
        "value": np.ascontiguousarray(inputs["value"][b]),
        "query": q,
        "refp": rp.reshape(LQP, 4 * NL).astype(np.float32),
        "consts": consts,
        "Wv": inputs["Wv"], "bvr": inputs["bv"][None, :],
        "Woff": inputs["Woff"], "boffr": inputs["boff"][None, :],
        "Watt": inputs["Watt"], "battr": inputs["batt"][None, :],
        "Wout": inputs["Wout"], "boutr": inputs["bout"][None, :],
    }


LAST_EXEC_NS = None


def kernel(**inputs):
    global LAST_EXEC_NS
    import os
    from concourse.bass_utils import run_bass_kernel_spmd
    nc = build_program(num_cores=8)
    in_maps = [prep_core_inputs(inputs, b) for b in range(8)]
    trace = bool(int(os.environ.get("DKA_TRACE", "0")))
    tdir = None
    if trace:
        tdir = "/tmp/dka_trace"
        os.makedirs(tdir, exist_ok=True)
    res = run_bass_kernel_spmd(nc, in_maps, core_ids=list(range(8)), trace=trace,
                               tmpdir=tdir)
    LAST_EXEC_NS = res.exec_time_ns
    return np.stack([res.results[b]["out"][:LQ] for b in range(8)], 0)


# revision 16
# speedup vs baseline: 1.3829x; 1.0742x over previous
# Multi-scale deformable attention kernel for TRN2 (per-core: one batch element).
#
# v3: per-level bf16 quad-corner M2 maps. Entry e of level l, head h =
#   [v_h(p), v_h(p+1), v_h(p+W), v_h(p+W+1)] (4x32 bf16 = 256B), p = LS_l+e-PAD_l
# so ONE 256B gather returns all 4 bilinear corners (order TL,TR,BL,BR).
# Build is fold-2: partition j holds entries 2j,2j+1 -> 512B-contiguous
# M2 writes at full DMA bandwidth. Levels built smallest-first; gather
# passes run per level (2,1,0) so the level-0 build overlaps the level-2/1
# gathers. Pool-engine descriptor generation (~5ns/idx) is the critical
# resource; the per-chunk coord/coef/idx vector work is emitted inside the
# level-2 pass so gather-buffer recycling never queues behind it.
import sys

sys.path.insert(0, "/opt/trn_rl_repo")
import numpy as np

import concourse.bacc as bacc
import concourse.bass as bass
import concourse.mybir as mybir
import concourse.tile as tile
import bass_rust
from concourse.alu_op_type import AluOpType
from concourse.masks import make_identity

F32 = mybir.dt.float32
BF16 = mybir.dt.bfloat16
I32 = mybir.dt.int32
I16 = mybir.dt.int16
AX = mybir.AxisListType
AF = mybir.ActivationFunctionType

SHAPES = ((100, 168), (50, 84), (25, 42))
NH, NL, NP = 8, 3, 4
P8 = 2 * NP              # 8 sampling points per (head, level)
C, D = 256, 32
W_ = [w for h, w in SHAPES]
H_ = [h for h, w in SHAPES]
LVL_START = [0, 16800, 21000]
L = 22050
PAD_L = [w + 2 for w in W_]                       # 170, 86, 44
NENT_L = [-(-(PAD_L[l] + H_[l] * W_[l] + 2) // 256) * 256 for l in range(NL)]
NR_L = [n // 256 for n in NENT_L]                 # 67, 17, 5 rounds
ESZ = 4 * D                                       # 128 bf16 per entry (256B)
HS_L = [n * ESZ for n in NENT_L]                  # head stride in bf16 elems
LQ = 1700
LQP = 1792               # 14 chunks of 128
NCH = LQP // 128
SLOTS = NH * NL * P8     # 192 (h,l,p) combos per query


def build_program(num_cores=8, dbg=False, mode='full'):
    nc = bacc.Bacc("TRN2", target_bir_lowering=False, debug=False,
                   num_devices=num_cores, num_swdge_queues=4)
    value = nc.dram_tensor("value", [L, 2 * C], F32, kind="ExternalInput")
    query = nc.dram_tensor("query", [LQP, C], F32, kind="ExternalInput")
    refp = nc.dram_tensor("refp", [LQP, 4 * NL], F32, kind="ExternalInput")
    consts = nc.dram_tensor("consts", [6 * SLOTS], F32, kind="ExternalInput")
    Wv = nc.dram_tensor("Wv", [2 * C, C], F32, kind="ExternalInput")
    bvr = nc.dram_tensor("bvr", [1, C], F32, kind="ExternalInput")
    Woff = nc.dram_tensor("Woff", [C, SLOTS * 2], F32, kind="ExternalInput")
    boffr = nc.dram_tensor("boffr", [1, SLOTS * 2], F32, kind="ExternalInput")
    Watt = nc.dram_tensor("Watt", [C, 96], F32, kind="ExternalInput")
    battr = nc.dram_tensor("battr", [1, 96], F32, kind="ExternalInput")
    Wout = nc.dram_tensor("Wout", [C, C], F32, kind="ExternalInput")
    boutr = nc.dram_tensor("boutr", [1, C], F32, kind="ExternalInput")
    out = nc.dram_tensor("out", [LQP, C], F32, kind="ExternalOutput")
    m2l = [nc.dram_tensor(f"m2{l}", [NH * HS_L[l]], BF16, kind="Internal")
           for l in range(NL)]

    from contextlib import ExitStack
    with tile.TileContext(nc) as tc:
      with ExitStack() as ctx:
        # ---------------- constant / parameter loads ----------------
        wp = ctx.enter_context(tc.tile_pool(name="wp", bufs=1))
        ident = wp.tile([128, 128], F32)
        make_identity(nc, ident[:])
        wv_t = [wp.tile([128, C], F32, tag=f"wv{k}", name=f"wv{k}") for k in range(4)]
        for k in range(4):
            nc.sync.dma_start(wv_t[k][:], Wv[128 * k:128 * (k + 1), :])
        woff_t = [wp.tile([128, SLOTS * 2], F32, tag=f"woff{k}", name=f"woff{k}") for k in range(2)]
        watt_t = [wp.tile([128, 96], F32, tag=f"watt{k}", name=f"watt{k}") for k in range(2)]
        wout_t = [wp.tile([128, C], F32, tag=f"wout{k}", name=f"wout{k}") for k in range(2)]
        for k in range(2):
            nc.sync.dma_start(woff_t[k][:], Woff[128 * k:128 * (k + 1), :])
            nc.sync.dma_start(watt_t[k][:], Watt[128 * k:128 * (k + 1), :])
            nc.sync.dma_start(wout_t[k][:], Wout[128 * k:128 * (k + 1), :])
        bv_t = wp.tile([1, C], F32)
        boff_t = wp.tile([1, SLOTS * 2], F32)
        batt_t = wp.tile([1, 96], F32)
        bout_t = wp.tile([1, C], F32)
        nc.sync.dma_start(bv_t[:], bvr[:])
        nc.sync.dma_start(boff_t[:], boffr[:])
        nc.sync.dma_start(batt_t[:], battr[:])
        nc.sync.dma_start(bout_t[:], boutr[:])
        ones_t = wp.tile([1, 128], F32)
        nc.gpsimd.memset(ones_t[:], 1.0)
        cst_row = wp.tile([1, 6 * SLOTS], F32)
        nc.sync.dma_start(cst_row[:], consts.ap().unsqueeze(0))
        cst = wp.tile([128, 6 * SLOTS], F32)
        nc.gpsimd.partition_broadcast(cst[:], cst_row[:])
        WT = cst[:, 0 * SLOTS:1 * SLOTS]
        W1 = cst[:, 1 * SLOTS:2 * SLOTS]
        W2 = cst[:, 2 * SLOTS:3 * SLOTS]
        H1 = cst[:, 3 * SLOTS:4 * SLOTS]
        H2 = cst[:, 4 * SLOTS:5 * SLOTS]
        BS = cst[:, 5 * SLOTS:6 * SLOTS]
        # bf16 copies: Wv, bias, ones, extraction matrices
        wvb = [wp.tile([128, C], BF16, tag=f"wvb{k}", name=f"wvb{k}") for k in range(4)]
        for k in range(4):
            nc.vector.tensor_copy(wvb[k][:], wv_t[k][:])
        bvb = wp.tile([1, C], BF16)
        nc.vector.tensor_copy(bvb[:], bv_t[:])
        onesb = wp.tile([1, 128], BF16)
        nc.gpsimd.memset(onesb[:], 1.0)
        # MB[par] = [zeros64 | M_par | zeros64], M_par[p,j] = 1 iff p = 2j+par.
        iv = ident[:].rearrange("p (j t) -> p j t", t=2)
        MB = [wp.tile([128, 192], BF16, tag=f"MB{p}", name=f"MB{p}") for p in range(2)]
        for p in range(2):
            nc.vector.memset(MB[p][:], 0.0)
            nc.vector.tensor_copy(MB[p][:, 64:128], iv[:, :, p])

        # ---------------- pools ----------------
        p1 = ctx.enter_context(tc.tile_pool(name="p1", bufs=3))
        vrp = ctx.enter_context(tc.tile_pool(name="vrp", bufs=8))
        ptd = ctx.enter_context(tc.tile_pool(name="ptd", bufs=2))
        psum = ctx.enter_context(tc.tile_pool(name="psum", bufs=1, space="PSUM"))
        p2 = ctx.enter_context(tc.tile_pool(name="p2", bufs=1))
        pwp = ctx.enter_context(tc.tile_pool(name="pwp", bufs=1))
        pg = ctx.enter_context(tc.tile_pool(name="pg", bufs=9))

        # ---------------- phase 1: per-level M2 build ----------------
        vtiles = {}

        def emit_proj(t):
            # project value rows [128t, 128t+128) -> bf16 v tile in the ring
            p0 = 128 * t
            vt = vrp.tile([128, C], BF16, tag="vring", name=f"vr{t}")
            rlo, rhi = max(0, -p0), min(128, L - p0)
            if rlo > 0 or rhi < 128:
                nc.vector.memset(vt[:], 0.0)
            if rhi > rlo:
                nr = rhi - rlo
                vin = p1.tile([128, 2 * C], F32, tag="vin")
                nc.sync.dma_start(vin[rlo:rhi, :], value[p0 + rlo:p0 + rhi, :])
                vT = p1.tile([128, 2 * C], BF16, tag="vT")
                for k in range(4):
                    pt = psum.tile([128, 128], F32, tag="tp", bufs=2, name="ptp")
                    nc.tensor.transpose(pt[:, 0:nr], vin[rlo:rhi, 128 * k:128 * (k + 1)],
                                        ident[0:nr, 0:nr])
                    nc.scalar.copy(vT[:, 128 * k:128 * k + nr], pt[:, 0:nr])
                ps = psum.tile([128, SLOTS * 2], F32, tag="mm", bufs=2, name="pmm")
                for k in range(4):
                    nc.tensor.matmul(ps[rlo:rhi, 0:C], vT[:, 128 * k:128 * k + nr],
                                     wvb[k][:], start=(k == 0), stop=False)
                nc.tensor.matmul(ps[rlo:rhi, 0:C], onesb[:, 0:nr], bvb[:],
                                 start=False, stop=True)
                nc.scalar.copy(vt[rlo:rhi, :], ps[rlo:rhi, 0:C])
            vtiles[t] = vt

        def emit_level_build(l):
            W, PADl, LS = W_[l], PAD_L[l], LVL_START[l]
            vtiles.clear()
            nxt = (LS - PADl) // 128
            # quad-corner extraction shifts: P0,P1,P2 then Q0,Q1,Q2 (=P+W)
            shifts = (0, 1, 2, W, W + 1, W + 2)
            for r in range(NR_L[l]):
                e0 = 256 * r
                need_hi = (LS - PADl + e0 + 255 + W + 2 + 1) // 128
                while nxt <= need_hi:
                    emit_proj(nxt)
                    nxt += 1
                # es cols [256*pi : 256*pi+256) = extraction pi
                es = psum.tile([128, 6 * C], F32, tag="es", bufs=1, name="es")
                for pi, sh in enumerate(shifts):
                    base = LS + e0 + sh - PADl
                    for jb in (0, 64):
                        P0 = base + 2 * jb
                        t = P0 // 128
                        s0 = P0 - 128 * t
                        colA, par2 = s0 // 2, s0 % 2
                        dst = es[jb:jb + 64, 256 * pi:256 * pi + 256]
                        if colA == 0:
                            nc.tensor.matmul(dst, MB[par2][:, 64:128], vtiles[t][:],
                                             start=True, stop=True)
                        else:
                            nc.tensor.matmul(dst, MB[par2][:, 64 + colA:128 + colA],
                                             vtiles[t][:], start=True, stop=False)
                            nc.tensor.matmul(dst, MB[par2][:, colA:colA + 64],
                                             vtiles[t + 1][:], start=False, stop=True)
                # assemble: partition j -> entries e0+2j, e0+2j+1, each
                # [P(e), P(e+1), Q(e), Q(e+1)]; extraction source per g-slot:
                TD = ptd.tile([128, 2048], BF16, tag="TD")
                TDv = TD[:].rearrange("p (h g d) -> p h g d", g=8, d=D)
                for gi, pi in enumerate((0, 1, 3, 4, 1, 2, 4, 5)):
                    nc.scalar.copy(
                        TDv[:, :, gi, :],
                        es[:, 256 * pi:256 * pi + 256].rearrange("p (h d) -> p h d", d=D))
                for h in range(NH):
                    seg = m2l[l].ap()[h * HS_L[l] + e0 * ESZ:
                                      h * HS_L[l] + (e0 + 256) * ESZ]
                    nc.sync.dma_start(seg.rearrange("(p c) -> p c", c=256),
                                      TD[:, 256 * h:256 * (h + 1)])

        # ---------------- phase 2 pre-work ----------------
        # A: matmul/softmax part (PE+ACT heavy) -- emitted early.
        # B: coords/coefs/idx part (DVE only) -- emitted inside the l=2 pass.
        rp_t, off_t, aw_t, wrp_t, coefx_t, O_t = {}, {}, {}, {}, {}, {}

        def emit_prework_a(ch):
            q0 = ch * 128
            qin = p2.tile([128, C], F32, tag="qin", bufs=2)
            nc.sync.dma_start(qin[:], query[q0:q0 + 128, :])
            rp = pwp.tile([128, 4 * NL], F32, tag=f"rp{ch}", name=f"rp{ch}")
            nc.sync.dma_start(rp[:], refp[q0:q0 + 128, :])
            qT = p2.tile([128, 256], F32, tag="qT", bufs=2)
            for k in range(2):
                pt2 = psum.tile([128, 128], F32, tag="tp", bufs=2, name="pt2")
                nc.tensor.transpose(pt2[:], qin[:, 128 * k:128 * (k + 1)], ident[:])
                nc.scalar.copy(qT[:, 128 * k:128 * (k + 1)], pt2[:])
            pso = psum.tile([128, SLOTS * 2], F32, tag="mm", bufs=2, name="pso")
            for k in range(2):
                nc.tensor.matmul(pso[:], qT[:, 128 * k:128 * (k + 1)], woff_t[k][:],
                                 start=(k == 0), stop=False)
            nc.tensor.matmul(pso[:], ones_t[:], boff_t[:], start=False, stop=True)
            off = pwp.tile([128, SLOTS * 2], F32, tag=f"off{ch}", name=f"off{ch}")
            nc.scalar.copy(off[:], pso[:])
            psa = psum.tile([128, SLOTS * 2], F32, tag="mm", bufs=2, name="psa")
            for k in range(2):
                nc.tensor.matmul(psa[:, 0:96], qT[:, 128 * k:128 * (k + 1)], watt_t[k][:],
                                 start=(k == 0), stop=False)
            nc.tensor.matmul(psa[:, 0:96], ones_t[:], batt_t[:], start=False, stop=True)
            att = p2.tile([128, 96], F32, tag="att", bufs=2)
            nc.scalar.copy(att[:], psa[:, 0:96])
            rmax = p2.tile([128, 8], F32, tag="rmax")
            nc.vector.tensor_reduce(rmax[:], att[:].rearrange("q (h l) -> q h l", l=12), AX.X, AluOpType.max)
            nc.vector.tensor_tensor(att[:].rearrange("q (h l) -> q h l", l=12),
                                    att[:].rearrange("q (h l) -> q h l", l=12),
                                    rmax[:].unsqueeze(2).broadcast_to((128, 8, 12)), AluOpType.subtract)
            nc.scalar.activation(att[:], att[:], AF.Exp)
            rsum = p2.tile([128, 8], F32, tag="rsum")
            nc.vector.tensor_reduce(rsum[:], att[:].rearrange("q (h l) -> q h l", l=12), AX.X, AluOpType.add)
            nc.vector.reciprocal(rsum[:], rsum[:])
            aw = pwp.tile([128, 96], F32, tag=f"aw{ch}", name=f"aw{ch}")
            nc.vector.tensor_tensor(aw[:].rearrange("q (h l) -> q h l", l=12),
                                    att[:].rearrange("q (h l) -> q h l", l=12),
                                    rsum[:].unsqueeze(2).broadcast_to((128, 8, 12)), AluOpType.mult)
            Ot = pwp.tile([128, C], F32, tag=f"O{ch}", name=f"O{ch}")
            rp_t[ch], off_t[ch], aw_t[ch], O_t[ch] = rp, off, aw, Ot

        def emit_prework_b(ch):
            rp, off, aw = rp_t[ch], off_t[ch], aw_t[ch]
            X = p2.tile([128, SLOTS], F32, tag="X")
            Y = p2.tile([128, SLOTS], F32, tag="Y")
            for du in range(2):
                for xy in range(2):
                    T = (X if xy == 0 else Y)
                    for li in range(NL):
                        dst = T[:].rearrange("q (hl p) -> q hl p", p=P8)[:, li::NL, du * NP:(du + 1) * NP]
                        src0 = off[:].rearrange("q (hl pc) -> q hl pc", pc=16)[:, li::NL, 2 * du + xy:2 * du + xy + 13:4]
                        src1 = rp[:, 4 * li + 2 * du + xy].unsqueeze(1).unsqueeze(2).broadcast_to((128, NH, NP))
                        nc.vector.scalar_tensor_tensor(dst, src0, -0.5, src1, AluOpType.add, AluOpType.add)
            TX = p2.tile([128, SLOTS], F32, tag="TX")
            TY = p2.tile([128, SLOTS], F32, tag="TY")
            X0 = p2.tile([128, SLOTS], F32, tag="X0")
            Y0 = p2.tile([128, SLOTS], F32, tag="Y0")
            MAGIC = 12582912.0  # 1.5 * 2^23: (x+M)-M = round-to-nearest(x)
            nc.vector.tensor_scalar(TX[:], X[:], MAGIC, MAGIC, AluOpType.add, AluOpType.subtract)
            nc.vector.tensor_scalar(TY[:], Y[:], MAGIC, MAGIC, AluOpType.add, AluOpType.subtract)
            nc.vector.tensor_tensor(X0[:], TX[:], X[:], AluOpType.is_gt)
            nc.vector.tensor_tensor(Y0[:], TY[:], Y[:], AluOpType.is_gt)
            nc.vector.tensor_tensor(X0[:], TX[:], X0[:], AluOpType.subtract)  # floor(x)
            nc.vector.tensor_tensor(Y0[:], TY[:], Y0[:], AluOpType.subtract)
            nc.vector.tensor_tensor(TX[:], X[:], X0[:], AluOpType.subtract)   # frac
            nc.vector.tensor_tensor(TY[:], Y[:], Y0[:], AluOpType.subtract)
            UX = p2.tile([128, SLOTS], F32, tag="UX")
            UY = p2.tile([128, SLOTS], F32, tag="UY")
            nc.vector.tensor_tensor(UX[:], W1, X0[:], AluOpType.subtract)   # W-1-x0
            nc.vector.tensor_tensor(UY[:], H1, Y0[:], AluOpType.subtract)
            MX0 = p2.tile([128, SLOTS], F32, tag="MX0")
            MY0 = p2.tile([128, SLOTS], F32, tag="MY0")
            MX1 = p2.tile([128, SLOTS], F32, tag="MX1")
            MY1 = p2.tile([128, SLOTS], F32, tag="MY1")
            nc.vector.tensor_tensor(MX0[:], X0[:], UX[:], AluOpType.min)
            nc.vector.tensor_tensor(MY0[:], Y0[:], UY[:], AluOpType.min)
            UX2 = p2.tile([128, SLOTS], F32, tag="UX2")
            UY2 = p2.tile([128, SLOTS], F32, tag="UY2")
            nc.vector.tensor_tensor(UX2[:], W2, X0[:], AluOpType.subtract)
            nc.vector.tensor_tensor(UY2[:], H2, Y0[:], AluOpType.subtract)
            nc.vector.scalar_tensor_tensor(MX1[:], X0[:], 1.0, UX2[:], AluOpType.add, AluOpType.min)
            nc.vector.scalar_tensor_tensor(MY1[:], Y0[:], 1.0, UY2[:], AluOpType.add, AluOpType.min)
            awsx = p2.tile([128, SLOTS], F32, tag="awsx")
            axv = awsx[:].rearrange("q (hl dp) -> q hl dp", dp=P8)
            avv = aw[:].rearrange("q (hl p) -> q hl p", p=NP)
            nc.vector.tensor_copy(axv[:, :, 0:NP], avv)
            nc.vector.tensor_copy(axv[:, :, NP:P8], avv)
            A = p2.tile([128, SLOTS], F32, tag="A")    # 1-tx
            B = p2.tile([128, SLOTS], F32, tag="B")    # 1-ty
            nc.vector.tensor_scalar(A[:], TX[:], -1.0, 1.0, AluOpType.mult, AluOpType.add)
            nc.vector.tensor_scalar(B[:], TY[:], -1.0, 1.0, AluOpType.mult, AluOpType.add)
            coefx = pwp.tile([128, SLOTS * 4], BF16, tag=f"coefx{ch}", name=f"coefx{ch}")
            cxv = coefx[:].rearrange("q (s c) -> q s c", c=4)
            vv = p2.tile([128, SLOTS], F32, tag="vv")
            wgt = p2.tile([128, SLOTS], F32, tag="wgt")
            # corner order matches quad entry: TL, TR, BL, BR
            for (ci, mx, my, wa, wb) in ((0, MX0, MY0, A, B), (1, MX1, MY0, TX, B),
                                         (2, MX0, MY1, A, TY), (3, MX1, MY1, TX, TY)):
                nc.vector.tensor_tensor(vv[:], mx[:], my[:], AluOpType.min)
                nc.vector.scalar_tensor_tensor(vv[:], vv[:], 0.0, awsx[:], AluOpType.is_ge, AluOpType.mult)
                nc.vector.tensor_tensor(wgt[:], wa[:], wb[:], AluOpType.mult)
                nc.vector.tensor_tensor(cxv[:, :, ci], wgt[:], vv[:], AluOpType.mult)
            X0C = p2.tile([128, SLOTS], F32, tag="X0C")
            Y0C = p2.tile([128, SLOTS], F32, tag="Y0C")
            nc.vector.scalar_tensor_tensor(X0C[:], X0[:], -1.0, W1, AluOpType.max, AluOpType.min)
            nc.vector.scalar_tensor_tensor(Y0C[:], Y0[:], -1.0, H1, AluOpType.max, AluOpType.min)
            IDXF = p2.tile([128, SLOTS], F32, tag="IDXF")
            nc.vector.tensor_tensor(IDXF[:], Y0C[:], WT, AluOpType.mult)
            nc.vector.tensor_tensor(IDXF[:], IDXF[:], X0C[:], AluOpType.add)
            nc.vector.tensor_tensor(IDXF[:], IDXF[:], BS, AluOpType.add)
            IDX32 = p2.tile([128, SLOTS], I32, tag="IDX32")
            nc.vector.tensor_copy(IDX32[:], IDXF[:])
            IDX16 = p2.tile([128, SLOTS], I16, tag="IDX16")
            nc.vector.tensor_copy(IDX16[:], IDX32[:])
            T16 = p2.tile([128, SLOTS], I16, tag="T16")
            nc.vector.stream_shuffle(T16[:], IDX16[:], [(i + 16) % 32 for i in range(32)])
            stage = p2.tile([128, SLOTS * 8], I16, tag="stage", bufs=2)
            nc.vector.memset(stage[0:32, :], 0)
            sv = stage[:].rearrange("p (s j) -> p s j", j=8)
            for k in range(4):
                nc.vector.tensor_copy(sv[0:16, :, 2 * k], IDX16[32 * k:32 * k + 16, :])
                nc.vector.tensor_copy(sv[0:16, :, 2 * k + 1], T16[32 * k:32 * k + 16, :])
            nc.vector.tensor_copy(stage[32:64, :], stage[0:32, :])
            nc.vector.tensor_copy(stage[64:96, :], stage[0:32, :])
            nc.vector.tensor_copy(stage[96:128, :], stage[0:32, :])
            wrp = pwp.tile([128, SLOTS * 8], I16, tag=f"wrp{ch}", name=f"wrp{ch}")
            nc.vector.stream_shuffle(wrp[:], stage[:], [i % 16 for i in range(32)])
            wrp_t[ch], coefx_t[ch] = wrp, coefx

        def emit_unit(l, ch, h):
            s0 = (h * NL + l) * P8        # first slot of (h,l) group
            G = pg.tile([128, P8 * ESZ], BF16, tag="G")
            m2ap = m2l[l].ap()
            m2ap.ap = bass_rust.VecI64Pair([[ESZ, NENT_L[l]], [1, ESZ]])
            m2ap.offset = h * HS_L[l]
            if mode == 'nog':
                nc.vector.memset(G[:], 0.01)
            else:
                nc.gpsimd.dma_gather(
                    G[:].rearrange("q (s e) -> q s e", e=ESZ), m2ap,
                    wrp_t[ch][:, 8 * s0:8 * s0 + 64], P8 * 128, P8 * 128, ESZ,
                    elem_step=ESZ, queue_num=(h * NL + l) % 4,
                    single_packet=False)
            cb = coefx_t[ch][:, 4 * s0:4 * s0 + 32].unsqueeze(2).broadcast_to((128, 32, 32))
            nc.vector.tensor_tensor(G[:].rearrange("q (sc c) -> q sc c", c=32),
                                    G[:].rearrange("q (sc c) -> q sc c", c=32),
                                    cb, AluOpType.mult)
            if l == 2:
                nc.vector.tensor_reduce(O_t[ch][:, D * h:D * (h + 1)],
                                        G[:].rearrange("q (sc c) -> q c sc", c=32),
                                        AX.X, AluOpType.add)
            else:
                Or = pg.tile([128, D], F32, tag="Or")
                nc.vector.tensor_reduce(Or[:],
                                        G[:].rearrange("q (sc c) -> q c sc", c=32),
                                        AX.X, AluOpType.add)
                nc.vector.tensor_tensor(O_t[ch][:, D * h:D * (h + 1)],
                                        O_t[ch][:, D * h:D * (h + 1)], Or[:],
                                        AluOpType.add)

        # ---------------- emission ----------------
        # level-2 map and the first chunks' prework go first so the first
        # gathers issue as early as possible; the big level-0 build overlaps
        # the level-2/1 gather passes.
        emit_level_build(2)
        for ch in range(4):
            emit_prework_a(ch)
        emit_prework_b(0)
        emit_prework_b(1)
        emit_level_build(1)
        for ch in range(4, NCH):
            emit_prework_a(ch)
        emit_level_build(0)
        # software-pipeline the DVE-side prework 3 chunks ahead of the
        # gathers so the Pool engine never waits on the coord/idx chain
        LOOK = 3
        for ch in range(2, min(LOOK, NCH)):
            emit_prework_b(ch)
        for ch in range(NCH):
            for h in range(NH):
                emit_unit(2, ch, h)
            if ch + LOOK < NCH:
                emit_prework_b(ch + LOOK)
        for l in (1, 0):
            for ch in range(NCH):
                for h in range(NH):
                    emit_unit(l, ch, h)

        # ---------------- out = O @ Wout + bout ----------------
        for ch in range(NCH):
            q0 = ch * 128
            OT = p2.tile([128, 256], F32, tag="OT", bufs=2)
            for k in range(2):
                pt3 = psum.tile([128, 128], F32, tag="tp", bufs=2, name="pt3")
                nc.tensor.transpose(pt3[:], O_t[ch][:, 128 * k:128 * (k + 1)], ident[:])
                nc.scalar.copy(OT[:, 128 * k:128 * (k + 1)], pt3[:])
            pso2 = psum.tile([128, SLOTS * 2], F32, tag="mm", bufs=2, name="pso2")
            for k in range(2):
                nc.tensor.matmul(pso2[:, 0:C], OT[:, 128 * k:128 * (k + 1)], wout_t[k][:],
                                 start=(k == 0), stop=False)
            nc.tensor.matmul(pso2[:, 0:C], ones_t[:], bout_t[:], start=False, stop=True)
            OO = p2.tile([128, C], F32, tag="OO", bufs=2)
            nc.scalar.copy(OO[:], pso2[:, 0:C])
            nc.sync.dma_start(out[q0:q0 + 128, :], OO[:])

    nc.finalize()
    return nc


# ---------------- host-side wrapper ----------------
def prep_core_inputs(inputs, b):
    q = np.zeros((LQP, C), np.float32)
    q[:LQ] = inputs["query"][b]
    rl = inputs["ref_l"][b].transpose(0, 2, 1, 3).reshape(LQ, NL, 2)
    rr = inputs["ref_r"][b].transpose(0, 2, 1, 3).reshape(LQ, NL, 2)
    norm = np.array([[w, h] for h, w in SHAPES], np.float32)
    rp = np.zeros((LQP, NL, 4), np.float32)
    rp[:LQ, :, 0:2] = rl * norm
    rp[:LQ, :, 2:4] = rr * norm
    slot_l = np.repeat(np.tile(np.arange(NL), NH), P8).astype(np.int32)  # slot -> level
    Wl = np.array(W_, np.float32)[slot_l]
    Hl = np.array(H_, np.float32)[slot_l]
    Bs = np.array(PAD_L, np.float32)[slot_l]       # per-level local base
    consts = np.concatenate([Wl, Wl - 1, Wl - 2, Hl - 1, Hl - 2, Bs]).astype(np.float32)
    return {
        "value": np.ascontiguousarray(inputs["value"][b]),
        "query": q,
        "refp": rp.reshape(LQP, 4 * NL).astype(np.float32),
        "consts": consts,
        "Wv": inputs["Wv"], "bvr": inputs["bv"][None, :],
        "Woff": inputs["Woff"], "boffr": inputs["boff"][None, :],
        "Watt": inputs["Watt"], "battr": inputs["batt"][None, :],
        "Wout": inputs["Wout"], "boutr": inputs["bout"][None, :],
    }


LAST_EXEC_NS = None


def kernel(**inputs):
    global LAST_EXEC_NS
    import os
    from concourse.bass_utils import run_bass_kernel_spmd
    nc = build_program(num_cores=8)
    in_maps = [prep_core_inputs(inputs, b) for b in range(8)]
    trace = bool(int(os.environ.get("DKA_TRACE", "0")))
    tdir = None
    if trace:
        tdir = "/tmp/dka_trace"
        os.makedirs(tdir, exist_ok=True)
    res = run_bass_kernel_spmd(nc, in_maps, core_ids=list(range(8)), trace=trace,
                               tmpdir=tdir)
    LAST_EXEC_NS = res.exec_time_ns
    return np.stack([res.results[b]["out"][:LQ] for b in range(8)], 0)
